# revision 12
# baseline (speedup 1.0000x reference)
"""Trainium2 Bass kernel for nn_DecoderLayer (self-attn + cross-attn + FFN, 3 LNs).

Sharding: 8 cores = 2 batches x 4 query-shards. Core c handles batch c//4 and
query blocks {q, q+4, q+8, q+12} (q = c%4, blocks of 128 rows) — stride-4 for
causal load balance with a padded-uniform suffix structure so all cores run the
same SPMD program. K/V projections are computed on contiguous 512-row shards
and exchanged with a single AllGather (self KV + cross KV together).

Layouts: activations feature-major (x.T: [d, seq] with d on partitions);
V position-major ([seq, dv]) so attn@V needs no transposes; scores computed
transposed ([kpos, q]) with softmax sums taken via an appended ones-column in
the V matmul. All matmuls run in float32r (full PE rate, ~1.5e-4 rel err).
"""
import os
import sys
import tempfile

import numpy as np

sys.path.insert(0, '/opt/trn_rl_repo')

import concourse.mybir as mybir  # noqa: E402
import concourse.tile as tile  # noqa: E402
from concourse import bacc, bass_utils  # noqa: E402

B, S, T, D, H, DK, DF = 2, 2048, 2048, 1024, 16, 64, 4096
EPS = 1e-5
NBLK = S // 128          # 16 k-blocks
NCH = D // 128           # 8 feature chunks
NPAIR = H // 2           # 8 head pairs
VW = DK + 1              # vf tile row width per head (ones column appended)
SEG = 4 * 512 * 1024     # per-rank AllGather elements
KOFF, VOFF, K2OFF, V2OFF = 0, 512 * 1024, 2 * 512 * 1024, 3 * 512 * 1024

F32 = mybir.dt.float32
F32R = mybir.dt.float32r
AF = mybir.ActivationFunctionType

_CACHE = {}


def _R(ap):
    return ap.bitcast(F32R)


def _sfx(kblk, causal):
    return 128 * (kblk // 4) if causal else 0


def _build(causal, affine):
    nc = bacc.Bacc("TRN2", target_bir_lowering=False, debug=False, num_devices=8)

    def mm(out, lhsT, rhs, **kw):
        nc.tensor.matmul(out, _R(lhsT), _R(rhs), **kw)

    def din(name, shape, dtype=F32):
        return nc.dram_tensor(name, shape, dtype, kind="ExternalInput").ap()

    xqT = din("xqT", [D, 512], F32R)
    xkT = din("xkT", [D, 512], F32R)
    encT = din("encT", [D, 512], F32R)
    W = {k: din("W" + k, [D, D], F32R) for k in ["q1", "k1", "v1", "o1", "q2", "k2", "v2", "o2"]}
    Wf1 = din("Wf1T", [D, DF], F32R)
    Wf2 = din("Wf2T", [DF, D], F32R)
    bias_in = {k: din("b" + k, [128, NCH]) for k in ["q1", "k1", "o1", "q2", "k2", "o2", "f2"]}
    bias_in["f1"] = din("bf1", [128, DF // 128])
    bv1 = din("bv1", [1, D])
    bv2 = din("bv2", [1, D])
    srcb = din("srcb", [128, NBLK])
    ones_in = din("ones_in", [128, H, 1], F32R)
    if causal:
        msk = din("mself", [NBLK, 128, 128], F32R)
    gb = {}
    if affine:
        for k in ["g1", "be1", "g2", "be2", "g3", "be3"]:
            gb[k] = din(k, [128, NCH])
    OUT = nc.dram_tensor("OUT", [D, 512], F32, kind="ExternalOutput").ap()

    CCIN = nc.dram_tensor("ccin", [SEG], F32R).ap()
    CCOUT = nc.dram_tensor("ccout", [4 * SEG], F32R).ap()

    with tile.TileContext(nc) as tc:
        with tc.tile_pool(name="const", bufs=1) as P_const, \
             tc.tile_pool(name="ps", bufs=4, space="PSUM") as ps, \
             tc.tile_pool(name="wpool", bufs=8) as P_w, \
             tc.tile_pool(name="ypool", bufs=1) as P_y:

            # ---- constants ----
            ones_t = P_const.tile([128, 1], F32R, tag="ones")
            nc.sync.dma_start(out=ones_t, in_=ones_in[:, 0, :])
            eps_t = P_const.tile([128, 1], F32, tag="eps")
            nc.vector.memset(eps_t, EPS)
            b_sb = {}
            for k, ap_ in bias_in.items():
                t = P_const.tile(list(ap_.shape), F32, tag="b" + k)
                nc.sync.dma_start(out=t, in_=ap_)
                b_sb[k] = t
            srcb_sb = P_const.tile([128, NBLK], F32, tag="srcb")
            nc.sync.dma_start(out=srcb_sb, in_=srcb)
            if causal:
                msk_sb = P_const.tile([128, NBLK, 128], F32R, tag="msk")
                nc.sync.dma_start(out=msk_sb, in_=msk.rearrange("k p q -> p k q"))
            gb_sb = {}
            if affine:
                for k in gb:
                    t = P_const.tile([128, NCH], F32, tag=k)
                    nc.sync.dma_start(out=t, in_=gb[k])
                    gb_sb[k] = t

            def wload(Wap):
                tiles = []
                for ki in range(NCH):
                    t = P_w.tile([128, Wap.shape[1]], F32R, tag="w")
                    nc.sync.dma_start(out=t, in_=Wap[ki * 128:(ki + 1) * 128, :])
                    tiles.append(t)
                return tiles

            def proj_feature(wt, rhs_tiles, bias_t, out_tiles, act=AF.Identity):
                for do in range(NCH):
                    p = ps.tile([128, 512], F32, tag="u")
                    for ki in range(NCH):
                        mm(p, wt[ki][:, do * 128:(do + 1) * 128],
                           rhs_tiles[ki], start=(ki == 0), stop=(ki == NCH - 1))
                    nc.scalar.activation(out=out_tiles[do], in_=p, func=act,
                                         bias=bias_t[:, do:do + 1], scale=1.0)

            # =========== phase 0: KV projections + AllGather + Q ===========
            with tc.tile_pool(name="xqpool", bufs=1) as P_xq:
                xq_t = []
                qT_t = []
                with tc.tile_pool(name="p0", bufs=1) as P0, \
                     tc.tile_pool(name="p0s", bufs=3) as P0s:
                    xk_t, enc_t = [], []
                    for ki in range(NCH):
                        t = P0.tile([128, 512], F32R, tag=f"xk{ki}")
                        nc.sync.dma_start(out=t, in_=xkT[ki * 128:(ki + 1) * 128, :])
                        xk_t.append(t)
                    for ki in range(NCH):
                        t = P0.tile([128, 512], F32R, tag=f"en{ki}")
                        nc.sync.dma_start(out=t, in_=encT[ki * 128:(ki + 1) * 128, :])
                        enc_t.append(t)
                    bvbc1 = P0.tile([128, D], F32, tag="bvbc1")
                    r1 = P0.tile([1, D], F32, tag="bvr1")
                    nc.sync.dma_start(out=r1, in_=bv1)
                    nc.gpsimd.partition_broadcast(bvbc1, r1)
                    bvbc2 = P0.tile([128, D], F32, tag="bvbc2")
                    r2 = P0.tile([1, D], F32, tag="bvr2")
                    nc.sync.dma_start(out=r2, in_=bv2)
                    nc.gpsimd.partition_broadcast(bvbc2, r2)

                    # K projections (feature-major) into CCIN
                    for (wkey, rhs, bkey, ccoff) in [("k1", xk_t, "k1", KOFF),
                                                     ("k2", enc_t, "k2", K2OFF)]:
                        wt = wload(W[wkey])
                        for do in range(NCH):
                            p = ps.tile([128, 512], F32, tag="u")
                            for ki in range(NCH):
                                mm(p, wt[ki][:, do * 128:(do + 1) * 128],
                                   rhs[ki], start=(ki == 0), stop=(ki == NCH - 1))
                            o = P0s.tile([128, 512], F32R, tag="kvo")
                            nc.scalar.activation(out=o, in_=p, func=AF.Identity,
                                                 bias=b_sb[bkey][:, do:do + 1], scale=1.0)
                            dst = CCIN[ccoff + do * 128 * 512: ccoff + (do + 1) * 128 * 512]
                            nc.sync.dma_start(out=dst.rearrange("(p s) -> p s", s=512), in_=o)

                    # V projections (position-major) into CCIN
                    for (wkey, lhs, bvbc, ccoff) in [("v1", xk_t, bvbc1, VOFF),
                                                     ("v2", enc_t, bvbc2, V2OFF)]:
                        wt = wload(W[wkey])
                        for sc in range(4):
                            p = ps.tile([128, D], F32, tag="u")
                            for ki in range(NCH):
                                for half in range(2):
                                    mm(p[:, half * 512:(half + 1) * 512],
                                       lhs[ki][:, sc * 128:(sc + 1) * 128],
                                       wt[ki][:, half * 512:(half + 1) * 512],
                                       start=(ki == 0), stop=(ki == NCH - 1))
                            o = P0s.tile([128, D], F32R, tag="kvo2")
                            nc.vector.tensor_add(o, p, bvbc)
                            dst = CCIN[ccoff + sc * 128 * D: ccoff + (sc + 1) * 128 * D]
                            nc.sync.dma_start(out=dst.rearrange("(p v) -> p v", v=D), in_=o)

                    nc.gpsimd.collective_compute(
                        "AllGather", mybir.AluOpType.bypass,
                        ins=[CCIN], outs=[CCOUT],
                        replica_groups=[[0, 1, 2, 3], [4, 5, 6, 7]],
                    )

                    # Q projection (overlaps the AllGather)
                    for ki in range(NCH):
                        t = P_xq.tile([128, 512], F32R, tag=f"xq{ki}")
                        nc.sync.dma_start(out=t, in_=xqT[ki * 128:(ki + 1) * 128, :])
                        xq_t.append(t)
                    qT_t = [P_xq.tile([128, 512], F32R, tag=f"q{i}", name=f"qT{i}") for i in range(NCH)]
                    proj_feature(wload(W["q1"]), xq_t, b_sb["q1"], qT_t)

                # ---- shared attention ----
                def attention(qtiles, koff, voff, causal_, use_srcb, out_pairs, Pstr):
                    for hp in range(NPAIR):
                        kt = Pstr.tile([128, 4, 512], F32R, tag="kt")
                        for r in range(4):
                            src = CCOUT[r * SEG + koff + hp * 128 * 512:
                                        r * SEG + koff + (hp + 1) * 128 * 512]
                            nc.sync.dma_start(out=kt[:, r, :],
                                              in_=src.rearrange("(p s) -> p s", s=512))
                        a0 = ps.tile([65, 512], F32, tag="u")
                        a1 = ps.tile([65, 512], F32, tag="u")
                        for kblk in range(NBLK):
                            sfx = _sfx(kblk, causal_)
                            if sfx >= 512:
                                continue
                            r, lb = kblk // 4, kblk % 4
                            vf = Pstr.tile([128, 2, VW], F32R, tag="vf")
                            vsrc = CCOUT[r * SEG + voff + lb * 128 * D:
                                         r * SEG + voff + (lb + 1) * 128 * D]
                            vsrc = vsrc.rearrange("(p h v) -> p h v", h=H, v=DK)
                            nc.sync.dma_start(out=vf[:, :, 0:DK],
                                              in_=vsrc[:, 2 * hp:2 * hp + 2, :])
                            nc.sync.dma_start(out=vf[:, :, DK:VW],
                                              in_=ones_in[:, 0:2, :])

                            sc_ps = ps.tile([128, 2, 512], F32, tag="u")
                            for h in range(2):
                                bp = h * DK
                                mm(sc_ps[:, h, sfx:512],
                                   kt[bp:bp + DK, r, lb * 128:lb * 128 + 128],
                                   qtiles[hp][bp:bp + DK, sfx:512],
                                   start=True, stop=True, tile_position=(bp, 0))
                            es = Pstr.tile([128, 2, 512], F32R, tag="es")
                            if use_srcb:
                                nc.scalar.activation(out=es[:, :, sfx:512],
                                                     in_=sc_ps[:, :, sfx:512],
                                                     func=AF.Exp, scale=0.125,
                                                     bias=srcb_sb[:, kblk:kblk + 1])
                            else:
                                nc.scalar.activation(out=es[:, :, sfx:512],
                                                     in_=sc_ps[:, :, sfx:512],
                                                     func=AF.Exp, scale=0.125)
                            if causal_:
                                for h in range(2):
                                    nc.vector.tensor_mul(es[:, h, sfx:sfx + 128],
                                                         es[:, h, sfx:sfx + 128],
                                                         msk_sb[:, kblk, :])
                            first, last = (kblk == 0), (kblk == NBLK - 1)
                            mm(a0[:, sfx:512], vf[:, 0, :], es[:, 0, sfx:512],
                               start=first, stop=last, skip_group_check=True)
                            mm(a1[:, sfx:512], vf[:, 1, :], es[:, 1, sfx:512],
                               start=first, stop=last, skip_group_check=True)
                        pair = out_pairs[hp]
                        for h, a in ((0, a0), (1, a1)):
                            rec = Pstr.tile([1, 512], F32, tag="rec")
                            nc.vector.reciprocal(rec, a[64:65, :])
                            bc = Pstr.tile([128, 512], F32, tag="bc")
                            nc.gpsimd.partition_broadcast(bc[0:DK, :], rec)
                            nc.vector.tensor_mul(pair[h * DK:(h + 1) * DK, :],
                                                 a[0:DK, :], bc[0:DK, :])

                def ln(z_tiles, gkey, bkey, out_tiles, Pstr):
                    st0 = ps.tile([1, 512], F32, tag="u")
                    st1 = ps.tile([1, 512], F32, tag="u")
                    for k in range(NCH):
                        mm(st0, ones_t, z_tiles[k],
                           start=(k == 0), stop=(k == NCH - 1), skip_group_check=True)
                    zsq = []
                    for k in range(NCH):
                        t = Pstr.tile([128, 512], F32R, tag="zsq")
                        nc.vector.tensor_mul(t, z_tiles[k], z_tiles[k])
                        zsq.append(t)
                    for k in range(NCH):
                        mm(st1, ones_t, zsq[k],
                           start=(k == 0), stop=(k == NCH - 1), skip_group_check=True)
                    mean = Pstr.tile([1, 512], F32, tag="lnrow")
                    nc.vector.tensor_scalar_mul(mean, st0, 1.0 / D)
                    var = Pstr.tile([1, 512], F32, tag="lnrow")
                    nc.vector.tensor_scalar_mul(var, st1, 1.0 / D)
                    msq = Pstr.tile([1, 512], F32, tag="lnrow")
                    nc.vector.tensor_mul(msq, mean, mean)
                    nc.vector.tensor_sub(var, var, msq)
                    sd = Pstr.tile([1, 512], F32, tag="lnrow")
                    nc.scalar.activation(out=sd, in_=var, func=AF.Sqrt,
                                         bias=eps_t[0:1, :], scale=1.0)
                    rstd = Pstr.tile([1, 512], F32, tag="lnrow")
                    nc.vector.reciprocal(rstd, sd)
                    nb = Pstr.tile([1, 512], F32, tag="lnrow")
                    nc.vector.tensor_mul(nb, mean, rstd)
                    nc.vector.tensor_scalar_mul(nb, nb, -1.0)
                    abc = Pstr.tile([128, 512], F32, tag="bc")
                    nc.gpsimd.partition_broadcast(abc, rstd)
                    bbc = Pstr.tile([128, 512], F32, tag="bc")
                    nc.gpsimd.partition_broadcast(bbc, nb)
                    for k in range(NCH):
                        t = Pstr.tile([128, 512], F32, tag="lnt")
                        nc.vector.tensor_mul(t, z_tiles[k], abc)
                        if affine:
                            t2 = Pstr.tile([128, 512], F32, tag="lnt")
                            nc.vector.tensor_add(t2, t, bbc)
                            nc.vector.tensor_scalar(out=out_tiles[k], in0=t2,
                                                    scalar1=gb_sb[gkey][:, k:k + 1],
                                                    scalar2=gb_sb[bkey][:, k:k + 1],
                                                    op0=mybir.AluOpType.mult,
                                                    op1=mybir.AluOpType.add)
                        else:
                            nc.vector.tensor_add(out_tiles[k], t, bbc)

                # =========== phase 1: self-attention + O1 + LN1 ===========
                attn_pairs = [P_y.tile([128, 512], F32R, tag=f"at{i}", name=f"atp{i}") for i in range(NPAIR)]
                y1_t = [P_y.tile([128, 512], F32R, tag=f"y{i}", name=f"y1t{i}") for i in range(NCH)]
                with tc.tile_pool(name="s1", bufs=3) as P_s1:
                    attention(qT_t, KOFF, VOFF, causal, False, attn_pairs, P_s1)
                    wt = wload(W["o1"])
                    for do in range(NCH):
                        p = ps.tile([128, 512], F32, tag="u")
                        for ki in range(NCH):
                            mm(p, wt[ki][:, do * 128:(do + 1) * 128],
                               attn_pairs[ki], start=(ki == 0), stop=(ki == NCH - 1))
                        o = P_s1.tile([128, 512], F32, tag="o1")
                        nc.scalar.activation(out=o, in_=p, func=AF.Identity,
                                             bias=b_sb["o1"][:, do:do + 1], scale=1.0)
                        # z1 in-place into xq tile (residual)
                        nc.vector.tensor_add(xq_t[do], o, xq_t[do])
                    ln(xq_t, "g1", "be1", y1_t, P_s1)

            # =========== phase 2: Q2 + cross-attention + O2 + LN2 ===========
            y2_t = [P_y.tile([128, 512], F32R, tag=f"y2{i}", name=f"y2t{i}") for i in range(NCH)]
            with tc.tile_pool(name="s2", bufs=3) as P_s2, \
                 tc.tile_pool(name="q2pool", bufs=1) as P_q2:
                q2_t = [P_q2.tile([128, 512], F32R, tag=f"qq{i}", name=f"q2t{i}") for i in range(NCH)]
                proj_feature(wload(W["q2"]), y1_t, b_sb["q2"], q2_t)
                attention(q2_t, K2OFF, V2OFF, False, True, attn_pairs, P_s2)
                wt = wload(W["o2"])
                for do in range(NCH):
                    p = ps.tile([128, 512], F32, tag="u")
                    for ki in range(NCH):
                        mm(p, wt[ki][:, do * 128:(do + 1) * 128],
                           attn_pairs[ki], start=(ki == 0), stop=(ki == NCH - 1))
                    o = P_s2.tile([128, 512], F32, tag="o2")
                    nc.scalar.activation(out=o, in_=p, func=AF.Identity,
                                         bias=b_sb["o2"][:, do:do + 1], scale=1.0)
                    nc.vector.tensor_add(y1_t[do], o, y1_t[do])  # z2 in-place
                ln(y1_t, "g2", "be2", y2_t, P_s2)

            # =========== phase 3: FFN + LN3 + output ===========
            with tc.tile_pool(name="s3", bufs=3) as P_s3, \
                 tc.tile_pool(name="hpool", bufs=2) as P_h, \
                 tc.tile_pool(name="holdpool", bufs=1) as P_hold:
                facc = [P_hold.tile([128, 512], F32, tag=f"fa{i}", name=f"facc{i}") for i in range(NCH)]
                for g in range(8):  # groups of 4 df-chunks
                    hg = []
                    for j in range(4):
                        dfc = g * 4 + j
                        p = ps.tile([128, 512], F32, tag="u")
                        for ki in range(NCH):
                            w = P_w.tile([128, 128], F32R, tag="wff")
                            nc.sync.dma_start(
                                out=w,
                                in_=Wf1[ki * 128:(ki + 1) * 128, dfc * 128:(dfc + 1) * 128])
                            mm(p, w, y2_t[ki], start=(ki == 0), stop=(ki == NCH - 1))
                        h = P_h.tile([128, 512], F32R, tag=f"h{j}")
                        nc.scalar.activation(out=h, in_=p, func=AF.Relu,
                                             bias=b_sb["f1"][:, dfc:dfc + 1], scale=1.0)
                        hg.append(h)
                    for do in range(NCH):
                        p2 = ps.tile([128, 512], F32, tag="u")
                        for j in range(4):
                            dfc = g * 4 + j
                            w2 = P_w.tile([128, 128], F32R, tag="wff")
                            nc.sync.dma_start(
                                out=w2,
                                in_=Wf2[dfc * 128:(dfc + 1) * 128, do * 128:(do + 1) * 128])
                            mm(p2, w2, hg[j], start=(j == 0), stop=(j == 3))
                        if g == 0:
                            f = facc[do]
                            nc.vector.tensor_scalar_add(f, p2, b_sb["f2"][:, do:do + 1])
                        else:
                            nc.vector.tensor_add(facc[do], facc[do], p2)
                y3_t = [P_hold.tile([128, 512], F32, tag=f"y3{i}", name=f"y3t{i}") for i in range(NCH)]
                for do in range(NCH):
                    nc.vector.tensor_add(y2_t[do], facc[do], y2_t[do])  # z3 in-place
                ln(y2_t, "g3", "be3", y3_t, P_s3)
                for k in range(NCH):
                    nc.sync.dma_start(out=OUT[k * 128:(k + 1) * 128, :], in_=y3_t[k])

    nc.compile()
    return nc


def _get_nc(causal, affine):
    key = (causal, affine)
    if key not in _CACHE:
        _CACHE[key] = _build(causal, affine)
    return _CACHE[key]


def kernel(**inputs):
    inp = {k: np.asarray(v) for k, v in inputs.items()}
    x, enc = inp['x'].astype(np.float32), inp['enc_out'].astype(np.float32)
    tgt = np.asarray(inp['tgt_mask'])[0, 0]
    src = np.asarray(inp['src_mask'])
    causal = bool((tgt == np.tril(np.ones((S, S), tgt.dtype))).all())
    if not causal and not bool((tgt != 0).all()):
        raise NotImplementedError("tgt_mask must be causal-tril or all-ones")
    affine = not (all((inp[f'g{i}'] == 1).all() for i in (1, 2, 3))
                  and all((inp[f'be{i}'] == 0).all() for i in (1, 2, 3)))

    WT = {k: np.ascontiguousarray(inp['W' + k].T) for k in
          ['q1', 'k1', 'v1', 'o1', 'q2', 'k2', 'v2', 'o2', 'f1', 'f2']}
    bch = {k: np.ascontiguousarray(inp['b' + k].reshape(-1, 128).T)
           for k in ['q1', 'k1', 'o1', 'q2', 'k2', 'o2', 'f1', 'f2']}

    nc = _get_nc(causal, affine)

    in_maps = []
    for c in range(8):
        b, q = c // 4, c % 4
        qblocks = [q + 4 * j for j in range(4)]
        qrows = np.concatenate([np.arange(g * 128, g * 128 + 128) for g in qblocks])
        m = {
            'xqT': np.ascontiguousarray(x[b, qrows].T),
            'xkT': np.ascontiguousarray(x[b, q * 512:(q + 1) * 512].T),
            'encT': np.ascontiguousarray(enc[b, q * 512:(q + 1) * 512].T),
            'bv1': np.ascontiguousarray(inp['bv1'][None, :]),
            'bv2': np.ascontiguousarray(inp['bv2'][None, :]),
            'ones_in': np.ones((128, H, 1), np.float32),
            'srcb': np.ascontiguousarray(
                np.where(src[b, 0, 0] == 0, np.float32(-1e9), np.float32(0.0))
                .astype(np.float32).reshape(NBLK, 128).T),
        }
        for k in ['q1', 'k1', 'v1', 'o1', 'q2', 'k2', 'v2', 'o2']:
            m['W' + k] = WT[k]
        m['Wf1T'] = WT['f1']
        m['Wf2T'] = WT['f2']
        for k in ['q1', 'k1', 'o1', 'q2', 'k2', 'o2', 'f2', 'f1']:
            m['b' + k] = bch[k]
        if causal:
            ms = np.empty((NBLK, 128, 128), np.float32)
            for kblk in range(NBLK):
                gq = qblocks[kblk // 4]
                ms[kblk] = tgt[gq * 128:(gq + 1) * 128,
                               kblk * 128:(kblk + 1) * 128].T.astype(np.float32)
            m['mself'] = np.ascontiguousarray(ms)
        if affine:
            for k in ['g1', 'be1', 'g2', 'be2', 'g3', 'be3']:
                m[k] = np.ascontiguousarray(inp[k].reshape(NCH, 128).T)
        in_maps.append(m)

    trace = bool(int(os.environ.get("KERNEL_TRACE", "0")))
    res = bass_utils.run_bass_kernel_spmd(
        nc, in_maps, core_ids=list(range(8)), trace=trace,
        tmpdir=(tempfile.mkdtemp(prefix="declayer_") if trace else None))
    kernel._last_results = res

    out = np.zeros((B, S, D), np.float32)
    for c in range(8):
        b, q = c // 4, c % 4
        qblocks = [q + 4 * j for j in range(4)]
        qrows = np.concatenate([np.arange(g * 128, g * 128 + 128) for g in qblocks])
        out[b, qrows] = res.results[c]['OUT'].T
    return out


# revision 20
# speedup vs baseline: 1.2372x; 1.2372x over previous
"""Trainium2 Bass kernel for nn_DecoderLayer (self-attn + cross-attn + FFN, 3 LNs).

Sharding: 8 cores = 2 batches x 4 query-shards. Core c handles batch c//4 and
query blocks {q, q+4, q+8, q+12} (q = c%4, blocks of 128 rows) — stride-4 for
causal load balance with a padded-uniform suffix structure so all cores run the
same SPMD program. K/V projections are computed on contiguous 512-row shards
and exchanged with a single AllGather (self KV + cross KV together).

Layouts: activations feature-major (x.T: [d, seq] with d on partitions);
V position-major ([seq, dv]) so attn@V needs no transposes; scores computed
transposed ([kpos, q]) with softmax sums taken via an appended ones-column in
the V matmul. All matmuls run in float32r (full PE rate, ~1.5e-4 rel err).
"""
import os
import sys
import tempfile

import numpy as np

sys.path.insert(0, '/opt/trn_rl_repo')

import concourse.mybir as mybir  # noqa: E402
import concourse.tile as tile  # noqa: E402
from concourse import bacc, bass_utils  # noqa: E402

B, S, T, D, H, DK, DF = 2, 2048, 2048, 1024, 16, 64, 4096
EPS = 1e-5
NBLK = S // 128          # 16 k-blocks
NCH = D // 128           # 8 feature chunks
NPAIR = H // 2           # 8 head pairs
VW = DK + 1              # V row width per head (ones column baked in)
KSEG = 1024 * 512
VSEG = 512 * H * VW
SEG = KSEG + VSEG        # per-rank elements of ONE AllGather (self or cross)

F32 = mybir.dt.float32
F32R = mybir.dt.float32r
AF = mybir.ActivationFunctionType

_CACHE = {}


def _R(ap):
    return ap.bitcast(F32R)


def _sfx(kblk, causal):
    return 128 * (kblk // 4) if causal else 0


def _build(causal, affine):
    nc = bacc.Bacc("TRN2", target_bir_lowering=False, debug=False, num_devices=8)

    def mm(out, lhsT, rhs, **kw):
        nc.tensor.matmul(out, _R(lhsT), _R(rhs), **kw)

    def din(name, shape, dtype=F32):
        return nc.dram_tensor(name, shape, dtype, kind="ExternalInput").ap()

    xqT = din("xqT", [D, 512], F32R)
    xkT = din("xkT", [D, 512], F32R)
    encT = din("encT", [D, 512], F32R)
    W = {k: din("W" + k, [D, D], F32R) for k in ["q1", "k1", "v1", "o1", "q2", "k2", "v2", "o2"]}
    Wf1 = din("Wf1T", [D, DF], F32R)
    Wf2 = din("Wf2T", [DF, D], F32R)
    bias_in = {k: din("b" + k, [128, NCH]) for k in ["q1", "k1", "o1", "q2", "k2", "o2", "f2"]}
    bias_in["f1"] = din("bf1", [128, DF // 128])
    bv1 = din("bv1", [1, D])
    bv2 = din("bv2", [1, D])
    srcb = din("srcb", [128, NBLK])
    ones_in = din("ones_in", [128, H, 1], F32R)
    if causal:
        msk = din("mself", [NBLK, 128, 128], F32R)
    gb = {}
    if affine:
        for k in ["g1", "be1", "g2", "be2", "g3", "be3"]:
            gb[k] = din(k, [128, NCH])
    OUT = nc.dram_tensor("OUT", [D, 512], F32, kind="ExternalOutput").ap()

    CCIN1 = nc.dram_tensor("ccin1", [SEG], F32R).ap()
    CCOUT1 = nc.dram_tensor("ccout1", [4 * SEG], F32R).ap()
    CCIN2 = nc.dram_tensor("ccin2", [SEG], F32R).ap()
    CCOUT2 = nc.dram_tensor("ccout2", [4 * SEG], F32R).ap()

    with tile.TileContext(nc) as tc:
        with tc.tile_pool(name="const", bufs=1) as P_const, \
             tc.tile_pool(name="ps", bufs=4, space="PSUM") as ps, \
             tc.tile_pool(name="ypool", bufs=1) as P_y:

            # ---- constants ----
            ones_t = P_const.tile([128, 1], F32R, tag="ones")
            nc.sync.dma_start(out=ones_t, in_=ones_in[:, 0, :])
            eps_t = P_const.tile([128, 1], F32, tag="eps")
            nc.vector.memset(eps_t, EPS)
            b_sb = {}
            for k, ap_ in bias_in.items():
                t = P_const.tile(list(ap_.shape), F32, tag="b" + k)
                nc.sync.dma_start(out=t, in_=ap_)
                b_sb[k] = t
            srcb_sb = P_const.tile([128, NBLK], F32, tag="srcb")
            nc.sync.dma_start(out=srcb_sb, in_=srcb)
            if causal:
                msk_sb = P_const.tile([128, NBLK, 128], F32R, tag="msk")
                nc.sync.dma_start(out=msk_sb, in_=msk.rearrange("k p q -> p k q"))
            gb_sb = {}
            if affine:
                for k in gb:
                    t = P_const.tile([128, NCH], F32, tag=k)
                    nc.sync.dma_start(out=t, in_=gb[k])
                    gb_sb[k] = t

            def wload(Wap, pool):
                tiles = []
                for ki in range(NCH):
                    t = pool.tile([128, Wap.shape[1]], F32R, tag="w", name=f"wt{ki}")
                    nc.gpsimd.dma_start(out=t, in_=Wap[ki * 128:(ki + 1) * 128, :])
                    tiles.append(t)
                return tiles

            def proj_feature(wt, rhs_tiles, bias_t, out_tiles, act=AF.Identity):
                for do in range(NCH):
                    p = ps.tile([128, 512], F32, tag="u")
                    for ki in range(NCH):
                        mm(p, wt[ki][:, do * 128:(do + 1) * 128],
                           rhs_tiles[ki], start=(ki == 0), stop=(ki == NCH - 1))
                    nc.scalar.activation(out=out_tiles[do], in_=p, func=act,
                                         bias=bias_t[:, do:do + 1], scale=1.0)

            # =========== phase 0: KV projections + AllGather + Q ===========
            with tc.tile_pool(name="xqpool", bufs=1) as P_xq:
                xq_t = []
                qT_t = []
                with tc.tile_pool(name="p0", bufs=1) as P0, \
                     tc.tile_pool(name="p0w", bufs=10) as P_w0, \
                     tc.tile_pool(name="p0s", bufs=3) as P0s:
                    onesbc = P0.tile([128, H, 1], F32R, tag="onesbc")
                    nc.sync.dma_start(out=onesbc, in_=ones_in)
                    xk_t, enc_t = [], []
                    for ki in range(NCH):
                        t = P0.tile([128, 512], F32R, tag=f"xk{ki}")
                        nc.sync.dma_start(out=t, in_=xkT[ki * 128:(ki + 1) * 128, :])
                        xk_t.append(t)
                    for ki in range(NCH):
                        t = P0.tile([128, 512], F32R, tag=f"en{ki}")
                        nc.sync.dma_start(out=t, in_=encT[ki * 128:(ki + 1) * 128, :])
                        enc_t.append(t)
                    bvbc1 = P0.tile([128, D], F32, tag="bvbc1")
                    r1 = P0.tile([1, D], F32, tag="bvr1")
                    nc.sync.dma_start(out=r1, in_=bv1)
                    nc.gpsimd.partition_broadcast(bvbc1, r1)
                    bvbc2 = P0.tile([128, D], F32, tag="bvbc2")
                    r2 = P0.tile([1, D], F32, tag="bvr2")
                    nc.sync.dma_start(out=r2, in_=bv2)
                    nc.gpsimd.partition_broadcast(bvbc2, r2)

                    def kproj(wkey, rhs, bkey, ccin):
                        wt = wload(W[wkey], P_w0)
                        for do in range(NCH):
                            p = ps.tile([128, 512], F32, tag="u")
                            for ki in range(NCH):
                                mm(p, wt[ki][:, do * 128:(do + 1) * 128],
                                   rhs[ki], start=(ki == 0), stop=(ki == NCH - 1))
                            o = P0s.tile([128, 512], F32R, tag="kvo")
                            nc.scalar.activation(out=o, in_=p, func=AF.Identity,
                                                 bias=b_sb[bkey][:, do:do + 1], scale=1.0)
                            dst = ccin[do * 128 * 512:(do + 1) * 128 * 512]
                            nc.sync.dma_start(out=dst.rearrange("(p s) -> p s", s=512), in_=o)

                    def vproj(wkey, lhs, bvbc, ccin):
                        wt = wload(W[wkey], P_w0)
                        for sc in range(4):
                            p = ps.tile([128, D], F32, tag="u")
                            for ki in range(NCH):
                                for half in range(2):
                                    mm(p[:, half * 512:(half + 1) * 512],
                                       lhs[ki][:, sc * 128:(sc + 1) * 128],
                                       wt[ki][:, half * 512:(half + 1) * 512],
                                       start=(ki == 0), stop=(ki == NCH - 1))
                            o = P0s.tile([128, H, VW], F32R, tag="kvo2")
                            nc.vector.tensor_add(
                                o[:, :, 0:DK],
                                p.rearrange("p (h v) -> p h v", v=DK),
                                bvbc.rearrange("p (h v) -> p h v", v=DK))
                            nc.vector.tensor_copy(o[:, :, DK:VW], onesbc)
                            dst = ccin[KSEG + sc * 128 * H * VW:
                                       KSEG + (sc + 1) * 128 * H * VW]
                            nc.sync.dma_start(
                                out=dst.rearrange("(p h v) -> p h v", h=H, v=VW), in_=o)

                    kproj("k1", xk_t, "k1", CCIN1)
                    vproj("v1", xk_t, bvbc1, CCIN1)
                    nc.gpsimd.collective_compute(
                        "AllGather", mybir.AluOpType.bypass,
                        ins=[CCIN1], outs=[CCOUT1],
                        replica_groups=[[0, 1, 2, 3], [4, 5, 6, 7]],
                    )
                    kproj("k2", enc_t, "k2", CCIN2)
                    vproj("v2", enc_t, bvbc2, CCIN2)
                    nc.gpsimd.collective_compute(
                        "AllGather", mybir.AluOpType.bypass,
                        ins=[CCIN2], outs=[CCOUT2],
                        replica_groups=[[0, 1, 2, 3], [4, 5, 6, 7]],
                    )

                    # Q projection (overlaps the AllGathers)
                    for ki in range(NCH):
                        t = P_xq.tile([128, 512], F32R, tag=f"xq{ki}")
                        nc.sync.dma_start(out=t, in_=xqT[ki * 128:(ki + 1) * 128, :])
                        xq_t.append(t)
                    qT_t = [P_xq.tile([128, 512], F32R, tag=f"q{i}", name=f"qT{i}") for i in range(NCH)]
                    proj_feature(wload(W["q1"], P_w0), xq_t, b_sb["q1"], qT_t)

                # ---- shared attention ----
                def attention(qtiles, ccout, causal_, use_srcb, out_pairs, Pstr):
                    for hp in range(NPAIR):
                        kt = Pstr.tile([128, 4, 512], F32R, tag="kt")
                        for r in range(4):
                            src = ccout[r * SEG + hp * 128 * 512:
                                        r * SEG + (hp + 1) * 128 * 512]
                            nc.scalar.dma_start(out=kt[:, r, :],
                                                in_=src.rearrange("(p s) -> p s", s=512))
                        a0 = ps.tile([65, 512], F32, tag="u")
                        a1 = ps.tile([65, 512], F32, tag="u")
                        for kblk in range(NBLK):
                            sfx = _sfx(kblk, causal_)
                            if sfx >= 512:
                                continue
                            r, lb = kblk // 4, kblk % 4
                            vf = Pstr.tile([128, 2, VW], F32R, tag="vf")
                            vsrc = ccout[r * SEG + KSEG + lb * 128 * H * VW:
                                         r * SEG + KSEG + (lb + 1) * 128 * H * VW]
                            vsrc = vsrc.rearrange("(p h v) -> p h v", h=H, v=VW)
                            nc.scalar.dma_start(out=vf,
                                                in_=vsrc[:, 2 * hp:2 * hp + 2, :])

                            sc_ps = ps.tile([128, 2, 512], F32, tag="u")
                            for h in range(2):
                                bp = h * DK
                                mm(sc_ps[:, h, sfx:512],
                                   kt[bp:bp + DK, r, lb * 128:lb * 128 + 128],
                                   qtiles[hp][bp:bp + DK, sfx:512],
                                   start=True, stop=True, tile_position=(bp, 0))
                            es = Pstr.tile([128, 2, 512], F32R, tag="es")
                            if use_srcb:
                                nc.scalar.activation(out=es[:, :, sfx:512],
                                                     in_=sc_ps[:, :, sfx:512],
                                                     func=AF.Exp, scale=0.125,
                                                     bias=srcb_sb[:, kblk:kblk + 1])
                            else:
                                nc.scalar.activation(out=es[:, :, sfx:512],
                                                     in_=sc_ps[:, :, sfx:512],
                                                     func=AF.Exp, scale=0.125)
                            if causal_:
                                for h in range(2):
                                    nc.vector.tensor_mul(es[:, h, sfx:sfx + 128],
                                                         es[:, h, sfx:sfx + 128],
                                                         msk_sb[:, kblk, :])
                            first, last = (kblk == 0), (kblk == NBLK - 1)
                            mm(a0[:, sfx:512], vf[:, 0, :], es[:, 0, sfx:512],
                               start=first, stop=last, skip_group_check=True)
                            mm(a1[:, sfx:512], vf[:, 1, :], es[:, 1, sfx:512],
                               start=first, stop=last, skip_group_check=True)
                        pair = out_pairs[hp]
                        for h, a in ((0, a0), (1, a1)):
                            rec = Pstr.tile([1, 512], F32, tag="rec")
                            nc.vector.reciprocal(rec, a[64:65, :])
                            bc = Pstr.tile([128, 512], F32, tag="bc")
                            nc.gpsimd.partition_broadcast(bc[0:DK, :], rec)
                            nc.vector.tensor_mul(pair[h * DK:(h + 1) * DK, :],
                                                 a[0:DK, :], bc[0:DK, :])

                def ln(z_tiles, gkey, bkey, out_tiles, Pstr):
                    st0 = ps.tile([1, 512], F32, tag="u")
                    st1 = ps.tile([1, 512], F32, tag="u")
                    for k in range(NCH):
                        mm(st0, ones_t, z_tiles[k],
                           start=(k == 0), stop=(k == NCH - 1), skip_group_check=True)
                    zsq = []
                    for k in range(NCH):
                        t = Pstr.tile([128, 512], F32R, tag="zsq")
                        nc.vector.tensor_mul(t, z_tiles[k], z_tiles[k])
                        zsq.append(t)
                    for k in range(NCH):
                        mm(st1, ones_t, zsq[k],
                           start=(k == 0), stop=(k == NCH - 1), skip_group_check=True)
                    mean = Pstr.tile([1, 512], F32, tag="lnrow")
                    nc.vector.tensor_scalar_mul(mean, st0, 1.0 / D)
                    var = Pstr.tile([1, 512], F32, tag="lnrow")
                    nc.vector.tensor_scalar_mul(var, st1, 1.0 / D)
                    msq = Pstr.tile([1, 512], F32, tag="lnrow")
                    nc.vector.tensor_mul(msq, mean, mean)
                    nc.vector.tensor_sub(var, var, msq)
                    sd = Pstr.tile([1, 512], F32, tag="lnrow")
                    nc.scalar.activation(out=sd, in_=var, func=AF.Sqrt,
                                         bias=eps_t[0:1, :], scale=1.0)
                    rstd = Pstr.tile([1, 512], F32, tag="lnrow")
                    nc.vector.reciprocal(rstd, sd)
                    nb = Pstr.tile([1, 512], F32, tag="lnrow")
                    nc.vector.tensor_mul(nb, mean, rstd)
                    nc.vector.tensor_scalar_mul(nb, nb, -1.0)
                    abc = Pstr.tile([128, 512], F32, tag="bc")
                    nc.gpsimd.partition_broadcast(abc, rstd)
                    bbc = Pstr.tile([128, 512], F32, tag="bc")
                    nc.gpsimd.partition_broadcast(bbc, nb)
                    for k in range(NCH):
                        t = Pstr.tile([128, 512], F32, tag="lnt")
                        nc.vector.tensor_mul(t, z_tiles[k], abc)
                        if affine:
                            t2 = Pstr.tile([128, 512], F32, tag="lnt")
                            nc.vector.tensor_add(t2, t, bbc)
                            nc.vector.tensor_scalar(out=out_tiles[k], in0=t2,
                                                    scalar1=gb_sb[gkey][:, k:k + 1],
                                                    scalar2=gb_sb[bkey][:, k:k + 1],
                                                    op0=mybir.AluOpType.mult,
                                                    op1=mybir.AluOpType.add)
                        else:
                            nc.vector.tensor_add(out_tiles[k], t, bbc)

                # =========== phase 1: self-attention + O1 + LN1 ===========
                attn_pairs = [P_y.tile([128, 512], F32R, tag=f"at{i}", name=f"atp{i}") for i in range(NPAIR)]
                y1_t = [P_y.tile([128, 512], F32R, tag=f"y{i}", name=f"y1t{i}") for i in range(NCH)]
                with tc.tile_pool(name="s1", bufs=3) as P_s1, \
                     tc.tile_pool(name="w1pool", bufs=10) as P_w1:
                    attention(qT_t, CCOUT1, causal, False, attn_pairs, P_s1)
                    wt = wload(W["o1"], P_w1)
                    for do in range(NCH):
                        p = ps.tile([128, 512], F32, tag="u")
                        for ki in range(NCH):
                            mm(p, wt[ki][:, do * 128:(do + 1) * 128],
                               attn_pairs[ki], start=(ki == 0), stop=(ki == NCH - 1))
                        o = P_s1.tile([128, 512], F32, tag="o1")
                        nc.scalar.activation(out=o, in_=p, func=AF.Identity,
                                             bias=b_sb["o1"][:, do:do + 1], scale=1.0)
                        # z1 in-place into xq tile (residual)
                        nc.vector.tensor_add(xq_t[do], o, xq_t[do])
                    ln(xq_t, "g1", "be1", y1_t, P_s1)

            # =========== phase 2: Q2 + cross-attention + O2 + LN2 ===========
            y2_t = [P_y.tile([128, 512], F32R, tag=f"y2{i}", name=f"y2t{i}") for i in range(NCH)]
            with tc.tile_pool(name="s2", bufs=3) as P_s2, \
                 tc.tile_pool(name="w2pool", bufs=10) as P_w2, \
                 tc.tile_pool(name="q2pool", bufs=1) as P_q2:
                q2_t = [P_q2.tile([128, 512], F32R, tag=f"qq{i}", name=f"q2t{i}") for i in range(NCH)]
                proj_feature(wload(W["q2"], P_w2), y1_t, b_sb["q2"], q2_t)
                attention(q2_t, CCOUT2, False, True, attn_pairs, P_s2)
                wt = wload(W["o2"], P_w2)
                for do in range(NCH):
                    p = ps.tile([128, 512], F32, tag="u")
                    for ki in range(NCH):
                        mm(p, wt[ki][:, do * 128:(do + 1) * 128],
                           attn_pairs[ki], start=(ki == 0), stop=(ki == NCH - 1))
                    o = P_s2.tile([128, 512], F32, tag="o2")
                    nc.scalar.activation(out=o, in_=p, func=AF.Identity,
                                         bias=b_sb["o2"][:, do:do + 1], scale=1.0)
                    nc.vector.tensor_add(y1_t[do], o, y1_t[do])  # z2 in-place
                ln(y1_t, "g2", "be2", y2_t, P_s2)

            # =========== phase 3: FFN + LN3 + output ===========
            with tc.tile_pool(name="s3", bufs=3) as P_s3, \
                 tc.tile_pool(name="hpool", bufs=2) as P_h, \
                 tc.tile_pool(name="wfpool", bufs=1) as P_wf, \
                 tc.tile_pool(name="holdpool", bufs=1) as P_hold:
                facc = [P_hold.tile([128, 512], F32, tag=f"fa{i}", name=f"facc{i}") for i in range(NCH)]
                for g in range(8):  # groups of 4 df-chunks
                    w1g = []
                    for ki in range(NCH):
                        w = P_wf.tile([128, 512], F32R, tag="w1", name=f"w1g{ki}", bufs=12)
                        nc.gpsimd.dma_start(
                            out=w, in_=Wf1[ki * 128:(ki + 1) * 128, g * 512:(g + 1) * 512])
                        w1g.append(w)
                    hg = []
                    for j in range(4):
                        dfc = g * 4 + j
                        p = ps.tile([128, 512], F32, tag="u")
                        for ki in range(NCH):
                            mm(p, w1g[ki][:, j * 128:(j + 1) * 128], y2_t[ki],
                               start=(ki == 0), stop=(ki == NCH - 1))
                        h = P_h.tile([128, 512], F32R, tag=f"h{j}")
                        nc.scalar.activation(out=h, in_=p, func=AF.Relu,
                                             bias=b_sb["f1"][:, dfc:dfc + 1], scale=1.0)
                        hg.append(h)
                    w2g = []
                    for j in range(4):
                        dfc = g * 4 + j
                        w = P_wf.tile([128, D], F32R, tag="w2", name=f"w2g{j}", bufs=6)
                        nc.gpsimd.dma_start(
                            out=w, in_=Wf2[dfc * 128:(dfc + 1) * 128, :])
                        w2g.append(w)
                    for do in range(NCH):
                        p2 = ps.tile([128, 512], F32, tag="u")
                        for j in range(4):
                            mm(p2, w2g[j][:, do * 128:(do + 1) * 128], hg[j],
                               start=(j == 0), stop=(j == 3))
                        if g == 0:
                            f = facc[do]
                            nc.vector.tensor_scalar_add(f, p2, b_sb["f2"][:, do:do + 1])
                        else:
                            nc.vector.tensor_add(facc[do], facc[do], p2)
                y3_t = [P_hold.tile([128, 512], F32, tag=f"y3{i}", name=f"y3t{i}") for i in range(NCH)]
                for do in range(NCH):
                    nc.vector.tensor_add(y2_t[do], facc[do], y2_t[do])  # z3 in-place
                ln(y2_t, "g3", "be3", y3_t, P_s3)
                for k in range(NCH):
                    nc.sync.dma_start(out=OUT[k * 128:(k + 1) * 128, :], in_=y3_t[k])

    nc.compile()
    return nc


def _get_nc(causal, affine):
    key = (causal, affine)
    if key not in _CACHE:
        _CACHE[key] = _build(causal, affine)
    return _CACHE[key]


def kernel(**inputs):
    inp = {k: np.asarray(v) for k, v in inputs.items()}
    x, enc = inp['x'].astype(np.float32), inp['enc_out'].astype(np.float32)
    tgt = np.asarray(inp['tgt_mask'])[0, 0]
    src = np.asarray(inp['src_mask'])
    causal = bool((tgt == np.tril(np.ones((S, S), tgt.dtype))).all())
    if not causal and not bool((tgt != 0).all()):
        raise NotImplementedError("tgt_mask must be causal-tril or all-ones")
    affine = not (all((inp[f'g{i}'] == 1).all() for i in (1, 2, 3))
                  and all((inp[f'be{i}'] == 0).all() for i in (1, 2, 3)))

    WT = {k: np.ascontiguousarray(inp['W' + k].T) for k in
          ['q1', 'k1', 'v1', 'o1', 'q2', 'k2', 'v2', 'o2', 'f1', 'f2']}
    bch = {k: np.ascontiguousarray(inp['b' + k].reshape(-1, 128).T)
           for k in ['q1', 'k1', 'o1', 'q2', 'k2', 'o2', 'f1', 'f2']}

    nc = _get_nc(causal, affine)

    in_maps = []
    for c in range(8):
        b, q = c // 4, c % 4
        qblocks = [q + 4 * j for j in range(4)]
        qrows = np.concatenate([np.arange(g * 128, g * 128 + 128) for g in qblocks])
        m = {
            'xqT': np.ascontiguousarray(x[b, qrows].T),
            'xkT': np.ascontiguousarray(x[b, q * 512:(q + 1) * 512].T),
            'encT': np.ascontiguousarray(enc[b, q * 512:(q + 1) * 512].T),
            'bv1': np.ascontiguousarray(inp['bv1'][None, :]),
            'bv2': np.ascontiguousarray(inp['bv2'][None, :]),
            'ones_in': np.ones((128, H, 1), np.float32),
            'srcb': np.ascontiguousarray(
                np.where(src[b, 0, 0] == 0, np.float32(-1e9), np.float32(0.0))
                .astype(np.float32).reshape(NBLK, 128).T),
        }
        for k in ['q1', 'k1', 'v1', 'o1', 'q2', 'k2', 'v2', 'o2']:
            m['W' + k] = WT[k]
        m['Wf1T'] = WT['f1']
        m['Wf2T'] = WT['f2']
        for k in ['q1', 'k1', 'o1', 'q2', 'k2', 'o2', 'f2', 'f1']:
            m['b' + k] = bch[k]
        if causal:
            ms = np.empty((NBLK, 128, 128), np.float32)
            for kblk in range(NBLK):
                gq = qblocks[kblk // 4]
                ms[kblk] = tgt[gq * 128:(gq + 1) * 128,
                               kblk * 128:(kblk + 1) * 128].T.astype(np.float32)
            m['mself'] = np.ascontiguousarray(ms)
        if affine:
            for k in ['g1', 'be1', 'g2', 'be2', 'g3', 'be3']:
                m[k] = np.ascontiguousarray(inp[k].reshape(NCH, 128).T)
        in_maps.append(m)

    trace = bool(int(os.environ.get("KERNEL_TRACE", "0")))
    res = bass_utils.run_bass_kernel_spmd(
        nc, in_maps, core_ids=list(range(8)), trace=trace,
        tmpdir=(tempfile.mkdtemp(prefix="declayer_") if trace else None))
    kernel._last_results = res

    out = np.zeros((B, S, D), np.float32)
    for c in range(8):
        b, q = c // 4, c % 4
        qblocks = [q + 4 * j for j in range(4)]
        qrows = np.concatenate([np.arange(g * 128, g * 128 + 128) for g in qblocks])
        out[b, qrows] = res.results[c]['OUT'].T
    return out


# revision 24
# speedup vs baseline: 1.5431x; 1.2472x over previous
"""Trainium2 Bass kernel for nn_DecoderLayer (self-attn + cross-attn + FFN, 3 LNs).

Sharding: 8 cores = 2 batches x 4 query-shards. Core c handles batch c//4 and
query blocks {q, q+4, q+8, q+12} (q = c%4, blocks of 128 rows) — stride-4 for
causal load balance with a padded-uniform suffix structure so all cores run the
same SPMD program. K/V projections are computed on contiguous 512-row shards
and exchanged with a single AllGather (self KV + cross KV together).

Layouts: activations feature-major (x.T: [d, seq] with d on partitions);
V position-major ([seq, dv]) so attn@V needs no transposes; scores computed
transposed ([kpos, q]) with softmax sums taken via an appended ones-column in
the V matmul. All matmuls run in float32r (full PE rate, ~1.5e-4 rel err).
"""
import os
import sys
import tempfile

import numpy as np

sys.path.insert(0, '/opt/trn_rl_repo')

import concourse.mybir as mybir  # noqa: E402
import concourse.tile as tile  # noqa: E402
from concourse import bacc, bass_utils  # noqa: E402

B, S, T, D, H, DK, DF = 2, 2048, 2048, 1024, 16, 64, 4096
EPS = 1e-5
NBLK = S // 128          # 16 k-blocks
NCH = D // 128           # 8 feature chunks
NPAIR = H // 2           # 8 head pairs
VW = DK + 1              # V row width per head (ones column baked in)
KSEG = 1024 * 512
VSEG = 512 * H * VW
SEG = KSEG + VSEG        # per-rank elements of ONE AllGather (self or cross)

F32 = mybir.dt.float32
F32R = mybir.dt.float32r
BF16 = mybir.dt.bfloat16
AF = mybir.ActivationFunctionType

_CACHE = {}


def _R(ap):
    return ap.bitcast(F32R) if ap.dtype == F32 else ap


def _sfx(kblk, causal):
    return 128 * (kblk // 4) if causal else 0


def _build(causal, affine):
    nc = bacc.Bacc("TRN2", target_bir_lowering=False, debug=False, num_devices=8)

    def mm(out, lhsT, rhs, **kw):
        nc.tensor.matmul(out, _R(lhsT), _R(rhs), **kw)

    def din(name, shape, dtype=F32):
        return nc.dram_tensor(name, shape, dtype, kind="ExternalInput").ap()

    xqT = din("xqT", [D, 512], F32R)
    xkT = din("xkT", [D, 512], F32R)
    encT = din("encT", [D, 512], F32R)
    W = {k: din("W" + k, [D, D], F32R) for k in ["q1", "k1", "v1", "o1", "q2", "k2", "v2", "o2"]}
    Wf1 = din("Wf1T", [D, DF], BF16)
    Wf2 = din("Wf2T", [DF, D], BF16)
    bias_in = {k: din("b" + k, [128, NCH]) for k in ["q1", "k1", "o1", "q2", "k2", "o2", "f2"]}
    bias_in["f1"] = din("bf1", [128, DF // 128])
    bv1 = din("bv1", [1, D])
    bv2 = din("bv2", [1, D])
    srcb = din("srcb", [128, NBLK])
    ones_in = din("ones_in", [128, H, 1], BF16)
    ones_f = din("ones_f", [128, 1], F32R)
    if causal:
        msk = din("mself", [NBLK, 128, 128], BF16)
    gb = {}
    if affine:
        for k in ["g1", "be1", "g2", "be2", "g3", "be3"]:
            gb[k] = din(k, [128, NCH])
    OUT = nc.dram_tensor("OUT", [D, 512], F32, kind="ExternalOutput").ap()

    CCIN1 = nc.dram_tensor("ccin1", [SEG], BF16).ap()
    CCOUT1 = nc.dram_tensor("ccout1", [4 * SEG], BF16).ap()
    CCIN2 = nc.dram_tensor("ccin2", [SEG], BF16).ap()
    CCOUT2 = nc.dram_tensor("ccout2", [4 * SEG], BF16).ap()

    with tile.TileContext(nc) as tc:
        with tc.tile_pool(name="const", bufs=1) as P_const, \
             tc.tile_pool(name="ps", bufs=4, space="PSUM") as ps, \
             tc.tile_pool(name="ypool", bufs=1) as P_y:

            # ---- constants ----
            ones_t = P_const.tile([128, 1], F32R, tag="ones")
            nc.sync.dma_start(out=ones_t, in_=ones_f)
            eps_t = P_const.tile([128, 1], F32, tag="eps")
            nc.vector.memset(eps_t, EPS)
            b_sb = {}
            for k, ap_ in bias_in.items():
                t = P_const.tile(list(ap_.shape), F32, tag="b" + k)
                nc.sync.dma_start(out=t, in_=ap_)
                b_sb[k] = t
            gb_sb = {}
            if affine:
                for k in gb:
                    t = P_const.tile([128, NCH], F32, tag=k)
                    nc.sync.dma_start(out=t, in_=gb[k])
                    gb_sb[k] = t

            def wload(Wap, pool):
                tiles = []
                for ki in range(NCH):
                    t = pool.tile([128, Wap.shape[1]], F32R, tag="w", name=f"wt{ki}")
                    nc.gpsimd.dma_start(out=t, in_=Wap[ki * 128:(ki + 1) * 128, :])
                    tiles.append(t)
                return tiles

            def proj_feature(wt, rhs_tiles, bias_t, out_tiles, act=AF.Identity):
                for do in range(NCH):
                    p = ps.tile([128, 512], F32, tag="u")
                    for ki in range(NCH):
                        mm(p, wt[ki][:, do * 128:(do + 1) * 128],
                           rhs_tiles[ki], start=(ki == 0), stop=(ki == NCH - 1))
                    nc.scalar.activation(out=out_tiles[do], in_=p, func=act,
                                         bias=bias_t[:, do:do + 1], scale=1.0)

            # =========== phase 0: KV projections + AllGather + Q ===========
            with tc.tile_pool(name="xqpool", bufs=1) as P_xq:
                xq_t = []
                qT_t = []
                with tc.tile_pool(name="p0", bufs=1) as P0, \
                     tc.tile_pool(name="p0w", bufs=10) as P_w0, \
                     tc.tile_pool(name="p0s", bufs=3) as P0s:
                    onesbc = P0.tile([128, H, 1], BF16, tag="onesbc")
                    nc.sync.dma_start(out=onesbc, in_=ones_in)
                    xk_t, enc_t = [], []
                    for ki in range(NCH):
                        t = P0.tile([128, 512], F32R, tag=f"xk{ki}")
                        nc.sync.dma_start(out=t, in_=xkT[ki * 128:(ki + 1) * 128, :])
                        xk_t.append(t)
                    for ki in range(NCH):
                        t = P0.tile([128, 512], F32R, tag=f"en{ki}")
                        nc.sync.dma_start(out=t, in_=encT[ki * 128:(ki + 1) * 128, :])
                        enc_t.append(t)
                    bvbc1 = P0.tile([128, D], F32, tag="bvbc1")
                    r1 = P0.tile([1, D], F32, tag="bvr1")
                    nc.sync.dma_start(out=r1, in_=bv1)
                    nc.gpsimd.partition_broadcast(bvbc1, r1)
                    bvbc2 = P0.tile([128, D], F32, tag="bvbc2")
                    r2 = P0.tile([1, D], F32, tag="bvr2")
                    nc.sync.dma_start(out=r2, in_=bv2)
                    nc.gpsimd.partition_broadcast(bvbc2, r2)

                    def kproj(wkey, rhs, bkey, ccin):
                        wt = wload(W[wkey], P_w0)
                        for do in range(NCH):
                            p = ps.tile([128, 512], F32, tag="u")
                            for ki in range(NCH):
                                mm(p, wt[ki][:, do * 128:(do + 1) * 128],
                                   rhs[ki], start=(ki == 0), stop=(ki == NCH - 1))
                            o = P0s.tile([128, 512], BF16, tag="kvo")
                            nc.scalar.activation(out=o, in_=p, func=AF.Identity,
                                                 bias=b_sb[bkey][:, do:do + 1], scale=1.0)
                            dst = ccin[do * 128 * 512:(do + 1) * 128 * 512]
                            nc.sync.dma_start(out=dst.rearrange("(p s) -> p s", s=512), in_=o)

                    def vproj(wkey, lhs, bvbc, ccin):
                        wt = wload(W[wkey], P_w0)
                        for sc in range(4):
                            p = ps.tile([128, D], F32, tag="u")
                            for ki in range(NCH):
                                for half in range(2):
                                    mm(p[:, half * 512:(half + 1) * 512],
                                       lhs[ki][:, sc * 128:(sc + 1) * 128],
                                       wt[ki][:, half * 512:(half + 1) * 512],
                                       start=(ki == 0), stop=(ki == NCH - 1))
                            o = P0s.tile([128, H, VW], BF16, tag="kvo2")
                            nc.vector.tensor_add(
                                o[:, :, 0:DK],
                                p.rearrange("p (h v) -> p h v", v=DK),
                                bvbc.rearrange("p (h v) -> p h v", v=DK))
                            nc.vector.tensor_copy(o[:, :, DK:VW], onesbc)
                            dst = ccin[KSEG + sc * 128 * H * VW:
                                       KSEG + (sc + 1) * 128 * H * VW]
                            nc.sync.dma_start(
                                out=dst.rearrange("(p h v) -> p h v", h=H, v=VW), in_=o)

                    kproj("k1", xk_t, "k1", CCIN1)
                    vproj("v1", xk_t, bvbc1, CCIN1)
                    nc.gpsimd.collective_compute(
                        "AllGather", mybir.AluOpType.bypass,
                        ins=[CCIN1], outs=[CCOUT1],
                        replica_groups=[[0, 1, 2, 3], [4, 5, 6, 7]],
                    )
                    kproj("k2", enc_t, "k2", CCIN2)
                    vproj("v2", enc_t, bvbc2, CCIN2)
                    nc.gpsimd.collective_compute(
                        "AllGather", mybir.AluOpType.bypass,
                        ins=[CCIN2], outs=[CCOUT2],
                        replica_groups=[[0, 1, 2, 3], [4, 5, 6, 7]],
                    )

                    # Q projection (overlaps the AllGathers)
                    for ki in range(NCH):
                        t = P_xq.tile([128, 512], F32R, tag=f"xq{ki}")
                        nc.sync.dma_start(out=t, in_=xqT[ki * 128:(ki + 1) * 128, :])
                        xq_t.append(t)
                    qT_t = [P_xq.tile([128, 512], BF16, tag=f"q{i}", name=f"qT{i}") for i in range(NCH)]
                    proj_feature(wload(W["q1"], P_w0), xq_t, b_sb["q1"], qT_t)

                # ---- shared attention ----
                def attention(qtiles, ccout, causal_, use_srcb, out_pairs, Pstr):
                    for hp in range(NPAIR):
                        kt = Pstr.tile([128, 4, 512], BF16, tag="kt")
                        for r in range(4):
                            src = ccout[r * SEG + hp * 128 * 512:
                                        r * SEG + (hp + 1) * 128 * 512]
                            nc.sync.dma_start(out=kt[:, r, :],
                                               in_=src.rearrange("(p s) -> p s", s=512))
                        a0 = ps.tile([65, 512], F32, tag="u")
                        a1 = ps.tile([65, 512], F32, tag="u")
                        for kblk in range(NBLK):
                            sfx = _sfx(kblk, causal_)
                            if sfx >= 512:
                                continue
                            r, lb = kblk // 4, kblk % 4
                            vf = Pstr.tile([128, 2, VW], BF16, tag="vf")
                            vsrc = ccout[r * SEG + KSEG + lb * 128 * H * VW:
                                         r * SEG + KSEG + (lb + 1) * 128 * H * VW]
                            vsrc = vsrc.rearrange("(p h v) -> p h v", h=H, v=VW)
                            nc.sync.dma_start(out=vf,
                                               in_=vsrc[:, 2 * hp:2 * hp + 2, :])

                            sc_ps = ps.tile([128, 2, 512], F32, tag="u")
                            for h in range(2):
                                bp = h * DK
                                mm(sc_ps[:, h, sfx:512],
                                   kt[bp:bp + DK, r, lb * 128:lb * 128 + 128],
                                   qtiles[hp][bp:bp + DK, sfx:512],
                                   start=True, stop=True, tile_position=(bp, 0))
                            es = Pstr.tile([128, 2, 512], BF16, tag="es")
                            if use_srcb:
                                nc.scalar.activation(out=es[:, :, sfx:512],
                                                     in_=sc_ps[:, :, sfx:512],
                                                     func=AF.Exp, scale=0.125,
                                                     bias=srcb_sb[:, kblk:kblk + 1])
                            else:
                                nc.scalar.activation(out=es[:, :, sfx:512],
                                                     in_=sc_ps[:, :, sfx:512],
                                                     func=AF.Exp, scale=0.125)
                            if causal_:
                                for h in range(2):
                                    nc.vector.tensor_mul(es[:, h, sfx:sfx + 128],
                                                         es[:, h, sfx:sfx + 128],
                                                         msk_sb[:, kblk, :])
                            first, last = (kblk == 0), (kblk == NBLK - 1)
                            mm(a0[:, sfx:512], vf[:, 0, :], es[:, 0, sfx:512],
                               start=first, stop=last, skip_group_check=True)
                            mm(a1[:, sfx:512], vf[:, 1, :], es[:, 1, sfx:512],
                               start=first, stop=last, skip_group_check=True)
                        pair = out_pairs[hp]
                        for h, a in ((0, a0), (1, a1)):
                            rec = Pstr.tile([1, 512], F32, tag="rec")
                            nc.vector.reciprocal(rec, a[64:65, :])
                            bc = Pstr.tile([128, 512], F32, tag="bc")
                            nc.gpsimd.partition_broadcast(bc[0:DK, :], rec)
                            nc.vector.tensor_mul(pair[h * DK:(h + 1) * DK, :],
                                                 a[0:DK, :], bc[0:DK, :])

                def ln(z_tiles, gkey, bkey, out_tiles, Pstr):
                    st0 = ps.tile([1, 512], F32, tag="u")
                    st1 = ps.tile([1, 512], F32, tag="u")
                    for k in range(NCH):
                        mm(st0, ones_t, z_tiles[k],
                           start=(k == 0), stop=(k == NCH - 1), skip_group_check=True)
                    zsq = []
                    for k in range(NCH):
                        t = Pstr.tile([128, 512], F32R, tag="zsq")
                        nc.vector.tensor_mul(t, z_tiles[k], z_tiles[k])
                        zsq.append(t)
                    for k in range(NCH):
                        mm(st1, ones_t, zsq[k],
                           start=(k == 0), stop=(k == NCH - 1), skip_group_check=True)
                    mean = Pstr.tile([1, 512], F32, tag="lnrow")
                    nc.vector.tensor_scalar_mul(mean, st0, 1.0 / D)
                    var = Pstr.tile([1, 512], F32, tag="lnrow")
                    nc.vector.tensor_scalar_mul(var, st1, 1.0 / D)
                    msq = Pstr.tile([1, 512], F32, tag="lnrow")
                    nc.vector.tensor_mul(msq, mean, mean)
                    nc.vector.tensor_sub(var, var, msq)
                    sd = Pstr.tile([1, 512], F32, tag="lnrow")
                    nc.scalar.activation(out=sd, in_=var, func=AF.Sqrt,
                                         bias=eps_t[0:1, :], scale=1.0)
                    rstd = Pstr.tile([1, 512], F32, tag="lnrow")
                    nc.vector.reciprocal(rstd, sd)
                    nb = Pstr.tile([1, 512], F32, tag="lnrow")
                    nc.vector.tensor_mul(nb, mean, rstd)
                    nc.vector.tensor_scalar_mul(nb, nb, -1.0)
                    abc = Pstr.tile([128, 512], F32, tag="bc")
                    nc.gpsimd.partition_broadcast(abc, rstd)
                    bbc = Pstr.tile([128, 512], F32, tag="bc")
                    nc.gpsimd.partition_broadcast(bbc, nb)
                    for k in range(NCH):
                        t = Pstr.tile([128, 512], F32, tag="lnt")
                        nc.vector.tensor_mul(t, z_tiles[k], abc)
                        if affine:
                            t2 = Pstr.tile([128, 512], F32, tag="lnt")
                            nc.vector.tensor_add(t2, t, bbc)
                            nc.vector.tensor_scalar(out=out_tiles[k], in0=t2,
                                                    scalar1=gb_sb[gkey][:, k:k + 1],
                                                    scalar2=gb_sb[bkey][:, k:k + 1],
                                                    op0=mybir.AluOpType.mult,
                                                    op1=mybir.AluOpType.add)
                        else:
                            nc.vector.tensor_add(out_tiles[k], t, bbc)

                # deferred const loads (keep startup DMA lean)
                srcb_sb = P_const.tile([128, NBLK], F32, tag="srcb")
                nc.sync.dma_start(out=srcb_sb, in_=srcb)
                msk_sb = None
                if causal:
                    msk_sb = P_const.tile([128, NBLK, 128], BF16, tag="msk")
                    nc.sync.dma_start(out=msk_sb, in_=msk.rearrange("k p q -> p k q"))

                # =========== phase 1: self-attention + O1 + LN1 ===========
                attn_pairs = [P_y.tile([128, 512], F32R, tag=f"at{i}", name=f"atp{i}") for i in range(NPAIR)]
                y1_t = [P_y.tile([128, 512], F32R, tag=f"y{i}", name=f"y1t{i}") for i in range(NCH)]
                with tc.tile_pool(name="s1", bufs=3) as P_s1, \
                     tc.tile_pool(name="w1pool", bufs=10) as P_w1:
                    attention(qT_t, CCOUT1, causal, False, attn_pairs, P_s1)
                    wt = wload(W["o1"], P_w1)
                    for do in range(NCH):
                        p = ps.tile([128, 512], F32, tag="u")
                        for ki in range(NCH):
                            mm(p, wt[ki][:, do * 128:(do + 1) * 128],
                               attn_pairs[ki], start=(ki == 0), stop=(ki == NCH - 1))
                        o = P_s1.tile([128, 512], F32, tag="o1")
                        nc.scalar.activation(out=o, in_=p, func=AF.Identity,
                                             bias=b_sb["o1"][:, do:do + 1], scale=1.0)
                        # z1 in-place into xq tile (residual)
                        nc.vector.tensor_add(xq_t[do], o, xq_t[do])
                    ln(xq_t, "g1", "be1", y1_t, P_s1)

            # =========== phase 2: Q2 + cross-attention + O2 + LN2 ===========
            y2_t = [P_y.tile([128, 512], F32R, tag=f"y2{i}", name=f"y2t{i}") for i in range(NCH)]
            with tc.tile_pool(name="s2", bufs=3) as P_s2, \
                 tc.tile_pool(name="w2pool", bufs=10) as P_w2, \
                 tc.tile_pool(name="q2pool", bufs=1) as P_q2:
                q2_t = [P_q2.tile([128, 512], BF16, tag=f"qq{i}", name=f"q2t{i}") for i in range(NCH)]
                proj_feature(wload(W["q2"], P_w2), y1_t, b_sb["q2"], q2_t)
                attention(q2_t, CCOUT2, False, True, attn_pairs, P_s2)
                wt = wload(W["o2"], P_w2)
                for do in range(NCH):
                    p = ps.tile([128, 512], F32, tag="u")
                    for ki in range(NCH):
                        mm(p, wt[ki][:, do * 128:(do + 1) * 128],
                           attn_pairs[ki], start=(ki == 0), stop=(ki == NCH - 1))
                    o = P_s2.tile([128, 512], F32, tag="o2")
                    nc.scalar.activation(out=o, in_=p, func=AF.Identity,
                                         bias=b_sb["o2"][:, do:do + 1], scale=1.0)
                    nc.vector.tensor_add(y1_t[do], o, y1_t[do])  # z2 in-place
                ln(y1_t, "g2", "be2", y2_t, P_s2)

            # =========== phase 3: FFN + LN3 + output ===========
            with tc.tile_pool(name="s3", bufs=3) as P_s3, \
                 tc.tile_pool(name="hpool", bufs=2) as P_h, \
                 tc.tile_pool(name="wfpool", bufs=1) as P_wf, \
                 tc.tile_pool(name="holdpool", bufs=1) as P_hold:
                facc = [P_hold.tile([128, 512], F32, tag=f"fa{i}", name=f"facc{i}") for i in range(NCH)]
                y2b = []
                for i in range(NCH):
                    t = P_hold.tile([128, 512], BF16, tag=f"y2b{i}", name=f"y2b{i}")
                    nc.vector.tensor_copy(t, y2_t[i])
                    y2b.append(t)
                for g in range(8):  # groups of 4 df-chunks
                    w1g = []
                    for ki in range(NCH):
                        w = P_wf.tile([128, 512], BF16, tag="w1", name=f"w1g{ki}", bufs=12)
                        nc.gpsimd.dma_start(
                            out=w, in_=Wf1[ki * 128:(ki + 1) * 128, g * 512:(g + 1) * 512])
                        w1g.append(w)
                    hg = []
                    for j in range(4):
                        dfc = g * 4 + j
                        p = ps.tile([128, 512], F32, tag="u")
                        for ki in range(NCH):
                            mm(p, w1g[ki][:, j * 128:(j + 1) * 128], y2b[ki],
                               start=(ki == 0), stop=(ki == NCH - 1))
                        h = P_h.tile([128, 512], BF16, tag=f"h{j}")
                        nc.scalar.activation(out=h, in_=p, func=AF.Relu,
                                             bias=b_sb["f1"][:, dfc:dfc + 1], scale=1.0)
                        hg.append(h)
                    w2g = []
                    for j in range(4):
                        dfc = g * 4 + j
                        w = P_wf.tile([128, D], BF16, tag="w2", name=f"w2g{j}", bufs=6)
                        nc.gpsimd.dma_start(
                            out=w, in_=Wf2[dfc * 128:(dfc + 1) * 128, :])
                        w2g.append(w)
                    for do in range(NCH):
                        p2 = ps.tile([128, 512], F32, tag="u")
                        for j in range(4):
                            mm(p2, w2g[j][:, do * 128:(do + 1) * 128], hg[j],
                               start=(j == 0), stop=(j == 3))
                        if g == 0:
                            f = facc[do]
                            nc.vector.tensor_scalar_add(f, p2, b_sb["f2"][:, do:do + 1])
                        else:
                            nc.vector.tensor_add(facc[do], facc[do], p2)
                y3_t = [P_hold.tile([128, 512], F32, tag=f"y3{i}", name=f"y3t{i}") for i in range(NCH)]
                for do in range(NCH):
                    nc.vector.tensor_add(y2_t[do], facc[do], y2_t[do])  # z3 in-place
                ln(y2_t, "g3", "be3", y3_t, P_s3)
                for k in range(NCH):
                    nc.sync.dma_start(out=OUT[k * 128:(k + 1) * 128, :], in_=y3_t[k])

    nc.compile()
    return nc


def _get_nc(causal, affine):
    key = (causal, affine)
    if key not in _CACHE:
        _CACHE[key] = _build(causal, affine)
    return _CACHE[key]


def kernel(**inputs):
    inp = {k: np.asarray(v) for k, v in inputs.items()}
    x, enc = inp['x'].astype(np.float32), inp['enc_out'].astype(np.float32)
    tgt = np.asarray(inp['tgt_mask'])[0, 0]
    src = np.asarray(inp['src_mask'])
    causal = bool((tgt == np.tril(np.ones((S, S), tgt.dtype))).all())
    if not causal and not bool((tgt != 0).all()):
        raise NotImplementedError("tgt_mask must be causal-tril or all-ones")
    affine = not (all((inp[f'g{i}'] == 1).all() for i in (1, 2, 3))
                  and all((inp[f'be{i}'] == 0).all() for i in (1, 2, 3)))

    import ml_dtypes
    BF = ml_dtypes.bfloat16
    WT = {k: np.ascontiguousarray(inp['W' + k].T) for k in
          ['q1', 'k1', 'v1', 'o1', 'q2', 'k2', 'v2', 'o2']}
    WT['f1'] = np.ascontiguousarray(inp['Wf1'].T.astype(BF))
    WT['f2'] = np.ascontiguousarray(inp['Wf2'].T.astype(BF))
    bch = {k: np.ascontiguousarray(inp['b' + k].reshape(-1, 128).T)
           for k in ['q1', 'k1', 'o1', 'q2', 'k2', 'o2', 'f1', 'f2']}

    nc = _get_nc(causal, affine)

    in_maps = []
    for c in range(8):
        b, q = c // 4, c % 4
        qblocks = [q + 4 * j for j in range(4)]
        qrows = np.concatenate([np.arange(g * 128, g * 128 + 128) for g in qblocks])
        m = {
            'xqT': np.ascontiguousarray(x[b, qrows].T),
            'xkT': np.ascontiguousarray(x[b, q * 512:(q + 1) * 512].T),
            'encT': np.ascontiguousarray(enc[b, q * 512:(q + 1) * 512].T),
            'bv1': np.ascontiguousarray(inp['bv1'][None, :]),
            'bv2': np.ascontiguousarray(inp['bv2'][None, :]),
            'ones_in': np.ones((128, H, 1), ml_dtypes.bfloat16),
            'ones_f': np.ones((128, 1), np.float32),
            'srcb': np.ascontiguousarray(
                np.where(src[b, 0, 0] == 0, np.float32(-1e9), np.float32(0.0))
                .astype(np.float32).reshape(NBLK, 128).T),
        }
        for k in ['q1', 'k1', 'v1', 'o1', 'q2', 'k2', 'v2', 'o2']:
            m['W' + k] = WT[k]
        m['Wf1T'] = WT['f1']
        m['Wf2T'] = WT['f2']
        for k in ['q1', 'k1', 'o1', 'q2', 'k2', 'o2', 'f2', 'f1']:
            m['b' + k] = bch[k]
        if causal:
            ms = np.empty((NBLK, 128, 128), np.float32)
            for kblk in range(NBLK):
                gq = qblocks[kblk // 4]
                ms[kblk] = tgt[gq * 128:(gq + 1) * 128,
                               kblk * 128:(kblk + 1) * 128].T.astype(np.float32)
            m['mself'] = np.ascontiguousarray(ms.astype(ml_dtypes.bfloat16))
        if affine:
            for k in ['g1', 'be1', 'g2', 'be2', 'g3', 'be3']:
                m[k] = np.ascontiguousarray(inp[k].reshape(NCH, 128).T)
        in_maps.append(m)

    trace = bool(int(os.environ.get("KERNEL_TRACE", "0")))
    res = bass_utils.run_bass_kernel_spmd(
        nc, in_maps, core_ids=list(range(8)), trace=trace,
        tmpdir=(tempfile.mkdtemp(prefix="declayer_") if trace else None))
    kernel._last_results = res

    out = np.zeros((B, S, D), np.float32)
    for c in range(8):
        b, q = c // 4, c % 4
        qblocks = [q + 4 * j for j in range(4)]
        qrows = np.concatenate([np.arange(g * 128, g * 128 + 128) for g in qblocks])
        out[b, qrows] = res.results[c]['OUT'].T
    return out


# revision 26
# speedup vs baseline: 1.6681x; 1.0810x over previous
"""Trainium2 Bass kernel for nn_DecoderLayer (self-attn + cross-attn + FFN, 3 LNs).

Sharding: 8 cores = 2 batches x 4 query-shards. Core c handles batch c//4 and
query blocks {q, q+4, q+8, q+12} (q = c%4, blocks of 128 rows) — stride-4 for
causal load balance with a padded-uniform suffix structure so all cores run the
same SPMD program. K/V projections are computed on contiguous 512-row shards
and exchanged with a single AllGather (self KV + cross KV together).

Layouts: activations feature-major (x.T: [d, seq] with d on partitions);
V position-major ([seq, dv]) so attn@V needs no transposes; scores computed
transposed ([kpos, q]) with softmax sums taken via an appended ones-column in
the V matmul. All matmuls run in float32r (full PE rate, ~1.5e-4 rel err).
"""
import os
import sys
import tempfile

import numpy as np

sys.path.insert(0, '/opt/trn_rl_repo')

import concourse.mybir as mybir  # noqa: E402
import concourse.tile as tile  # noqa: E402
from concourse import bacc, bass_utils  # noqa: E402

B, S, T, D, H, DK, DF = 2, 2048, 2048, 1024, 16, 64, 4096
EPS = 1e-5
NBLK = S // 128          # 16 k-blocks
NCH = D // 128           # 8 feature chunks
NPAIR = H // 2           # 8 head pairs
VW = DK + 1              # V row width per head (ones column baked in)
KSEG = 1024 * 512
VSEG = 512 * H * VW
SEG = KSEG + VSEG        # per-rank elements of ONE AllGather (self or cross)

F32 = mybir.dt.float32
F32R = mybir.dt.float32r
BF16 = mybir.dt.bfloat16
AF = mybir.ActivationFunctionType

_CACHE = {}


def _R(ap):
    return ap.bitcast(F32R) if ap.dtype == F32 else ap


def _sfx(kblk, causal):
    return 128 * (kblk // 4) if causal else 0


def _build(causal, affine):
    nc = bacc.Bacc("TRN2", target_bir_lowering=False, debug=False, num_devices=8)

    def mm(out, lhsT, rhs, **kw):
        nc.tensor.matmul(out, _R(lhsT), _R(rhs), **kw)

    def din(name, shape, dtype=F32):
        return nc.dram_tensor(name, shape, dtype, kind="ExternalInput").ap()

    xqT = din("xqT", [D, 512], F32R)
    xkT = din("xkT", [D, 512], F32R)
    encT = din("encT", [D, 512], F32R)
    W = {k: din("W" + k, [D, D], F32R) for k in ["q1", "k1", "v1", "o1", "q2", "k2", "v2", "o2"]}
    Wf1 = din("Wf1T", [D, DF], BF16)
    Wf2 = din("Wf2T", [DF, D], BF16)
    bias_in = {k: din("b" + k, [128, NCH]) for k in ["q1", "k1", "o1", "q2", "k2", "o2", "f2"]}
    bias_in["f1"] = din("bf1", [128, DF // 128])
    bv1 = din("bv1", [1, D])
    bv2 = din("bv2", [1, D])
    srcb = din("srcb", [128, NBLK])
    ones_in = din("ones_in", [128, H, 1], BF16)
    ones_f = din("ones_f", [128, 1], F32R)
    if causal:
        msk = din("mself", [NBLK, 128, 128], BF16)
    gb = {}
    if affine:
        for k in ["g1", "be1", "g2", "be2", "g3", "be3"]:
            gb[k] = din(k, [128, NCH])
    OUT = nc.dram_tensor("OUT", [D, 512], F32, kind="ExternalOutput").ap()

    CCIN1 = nc.dram_tensor("ccin1", [SEG], BF16).ap()
    CCOUT1 = nc.dram_tensor("ccout1", [4 * SEG], BF16).ap()
    CCIN2 = nc.dram_tensor("ccin2", [SEG], BF16).ap()
    CCOUT2 = nc.dram_tensor("ccout2", [4 * SEG], BF16).ap()

    with tile.TileContext(nc) as tc:
        with tc.tile_pool(name="const", bufs=1) as P_const, \
             tc.tile_pool(name="ps", bufs=3, space="PSUM") as ps, \
             tc.tile_pool(name="psatt", bufs=2, space="PSUM") as ps_att, \
             tc.tile_pool(name="ypool", bufs=1) as P_y:

            # ---- constants ----
            ones_t = P_const.tile([128, 1], F32R, tag="ones")
            nc.sync.dma_start(out=ones_t, in_=ones_f)
            eps_t = P_const.tile([128, 1], F32, tag="eps")
            nc.vector.memset(eps_t, EPS)
            b_sb = {}
            for k, ap_ in bias_in.items():
                t = P_const.tile(list(ap_.shape), F32, tag="b" + k)
                nc.sync.dma_start(out=t, in_=ap_)
                b_sb[k] = t
            gb_sb = {}
            if affine:
                for k in gb:
                    t = P_const.tile([128, NCH], F32, tag=k)
                    nc.sync.dma_start(out=t, in_=gb[k])
                    gb_sb[k] = t

            def wload(Wap, pool):
                tiles = []
                for ki in range(NCH):
                    t = pool.tile([128, Wap.shape[1]], F32R, tag="w", name=f"wt{ki}")
                    nc.gpsimd.dma_start(out=t, in_=Wap[ki * 128:(ki + 1) * 128, :])
                    tiles.append(t)
                return tiles

            def proj_feature(wt, rhs_tiles, bias_t, out_tiles, act=AF.Identity):
                for do in range(NCH):
                    p = ps.tile([128, 512], F32, tag="u")
                    for ki in range(NCH):
                        mm(p, wt[ki][:, do * 128:(do + 1) * 128],
                           rhs_tiles[ki], start=(ki == 0), stop=(ki == NCH - 1))
                    nc.scalar.activation(out=out_tiles[do], in_=p, func=act,
                                         bias=bias_t[:, do:do + 1], scale=1.0)

            # =========== phase 0: KV projections + AllGather + Q ===========
            with tc.tile_pool(name="xqpool", bufs=1) as P_xq:
                xq_t = []
                qT_t = []
                with tc.tile_pool(name="p0", bufs=1) as P0, \
                     tc.tile_pool(name="p0w", bufs=10) as P_w0, \
                     tc.tile_pool(name="p0s", bufs=3) as P0s:
                    onesbc = P0.tile([128, H, 1], BF16, tag="onesbc")
                    nc.sync.dma_start(out=onesbc, in_=ones_in)
                    xk_t, enc_t = [], []
                    for ki in range(NCH):
                        t = P0.tile([128, 512], F32R, tag=f"xk{ki}")
                        nc.sync.dma_start(out=t, in_=xkT[ki * 128:(ki + 1) * 128, :])
                        xk_t.append(t)
                    for ki in range(NCH):
                        t = P0.tile([128, 512], F32R, tag=f"en{ki}")
                        nc.sync.dma_start(out=t, in_=encT[ki * 128:(ki + 1) * 128, :])
                        enc_t.append(t)
                    bvbc1 = P0.tile([128, D], F32, tag="bvbc1")
                    r1 = P0.tile([1, D], F32, tag="bvr1")
                    nc.sync.dma_start(out=r1, in_=bv1)
                    nc.gpsimd.partition_broadcast(bvbc1, r1)
                    bvbc2 = P0.tile([128, D], F32, tag="bvbc2")
                    r2 = P0.tile([1, D], F32, tag="bvr2")
                    nc.sync.dma_start(out=r2, in_=bv2)
                    nc.gpsimd.partition_broadcast(bvbc2, r2)

                    def kproj(wkey, rhs, bkey, ccin):
                        wt = wload(W[wkey], P_w0)
                        for do in range(NCH):
                            p = ps.tile([128, 512], F32, tag="u")
                            for ki in range(NCH):
                                mm(p, wt[ki][:, do * 128:(do + 1) * 128],
                                   rhs[ki], start=(ki == 0), stop=(ki == NCH - 1))
                            o = P0s.tile([128, 512], BF16, tag="kvo")
                            nc.scalar.activation(out=o, in_=p, func=AF.Identity,
                                                 bias=b_sb[bkey][:, do:do + 1], scale=1.0)
                            dst = ccin[do * 128 * 512:(do + 1) * 128 * 512]
                            nc.sync.dma_start(out=dst.rearrange("(p s) -> p s", s=512), in_=o)

                    def vproj(wkey, lhs, bvbc, ccin):
                        wt = wload(W[wkey], P_w0)
                        for sc in range(4):
                            p = ps.tile([128, D], F32, tag="u")
                            for ki in range(NCH):
                                for half in range(2):
                                    mm(p[:, half * 512:(half + 1) * 512],
                                       lhs[ki][:, sc * 128:(sc + 1) * 128],
                                       wt[ki][:, half * 512:(half + 1) * 512],
                                       start=(ki == 0), stop=(ki == NCH - 1))
                            o = P0s.tile([128, H, VW], BF16, tag="kvo2")
                            nc.vector.tensor_add(
                                o[:, :, 0:DK],
                                p.rearrange("p (h v) -> p h v", v=DK),
                                bvbc.rearrange("p (h v) -> p h v", v=DK))
                            nc.vector.tensor_copy(o[:, :, DK:VW], onesbc)
                            dst = ccin[KSEG + sc * 128 * H * VW:
                                       KSEG + (sc + 1) * 128 * H * VW]
                            nc.sync.dma_start(
                                out=dst.rearrange("(p h v) -> p h v", h=H, v=VW), in_=o)

                    kproj("k1", xk_t, "k1", CCIN1)
                    vproj("v1", xk_t, bvbc1, CCIN1)
                    nc.gpsimd.collective_compute(
                        "AllGather", mybir.AluOpType.bypass,
                        ins=[CCIN1], outs=[CCOUT1],
                        replica_groups=[[0, 1, 2, 3], [4, 5, 6, 7]],
                    )
                    kproj("k2", enc_t, "k2", CCIN2)
                    vproj("v2", enc_t, bvbc2, CCIN2)
                    nc.gpsimd.collective_compute(
                        "AllGather", mybir.AluOpType.bypass,
                        ins=[CCIN2], outs=[CCOUT2],
                        replica_groups=[[0, 1, 2, 3], [4, 5, 6, 7]],
                    )

                    # Q projection (overlaps the AllGathers)
                    for ki in range(NCH):
                        t = P_xq.tile([128, 512], F32R, tag=f"xq{ki}")
                        nc.sync.dma_start(out=t, in_=xqT[ki * 128:(ki + 1) * 128, :])
                        xq_t.append(t)
                    qT_t = [P_xq.tile([128, 512], BF16, tag=f"q{i}", name=f"qT{i}") for i in range(NCH)]
                    proj_feature(wload(W["q1"], P_w0), xq_t, b_sb["q1"], qT_t)

                # ---- shared attention ----
                def attention(qtiles, ccout, causal_, use_srcb, out_pairs, Pstr):
                    for hp in range(NPAIR):
                        kt = Pstr.tile([128, 4, 512], BF16, tag="kt")
                        for r in range(4):
                            src = ccout[r * SEG + hp * 128 * 512:
                                        r * SEG + (hp + 1) * 128 * 512]
                            nc.sync.dma_start(out=kt[:, r, :],
                                               in_=src.rearrange("(p s) -> p s", s=512))
                        a0 = ps_att.tile([65, 512], F32, tag="a")
                        a1 = ps_att.tile([65, 512], F32, tag="a")
                        for kblk in range(NBLK):
                            sfx = _sfx(kblk, causal_)
                            if sfx >= 512:
                                continue
                            r, lb = kblk // 4, kblk % 4
                            vf = Pstr.tile([128, 2, VW], BF16, tag="vf")
                            vsrc = ccout[r * SEG + KSEG + lb * 128 * H * VW:
                                         r * SEG + KSEG + (lb + 1) * 128 * H * VW]
                            vsrc = vsrc.rearrange("(p h v) -> p h v", h=H, v=VW)
                            nc.sync.dma_start(out=vf,
                                               in_=vsrc[:, 2 * hp:2 * hp + 2, :])

                            sc_ps = ps.tile([128, 2, 512], F32, tag="u")
                            for h in range(2):
                                bp = h * DK
                                mm(sc_ps[:, h, sfx:512],
                                   kt[bp:bp + DK, r, lb * 128:lb * 128 + 128],
                                   qtiles[hp][bp:bp + DK, sfx:512],
                                   start=True, stop=True, tile_position=(bp, 0))
                            es = Pstr.tile([128, 2, 512], BF16, tag="es")
                            if use_srcb:
                                nc.scalar.activation(out=es[:, :, sfx:512],
                                                     in_=sc_ps[:, :, sfx:512],
                                                     func=AF.Exp, scale=0.125,
                                                     bias=srcb_sb[:, kblk:kblk + 1])
                            else:
                                nc.scalar.activation(out=es[:, :, sfx:512],
                                                     in_=sc_ps[:, :, sfx:512],
                                                     func=AF.Exp, scale=0.125)
                            if causal_:
                                nc.vector.tensor_mul(
                                    es[:, :, sfx:sfx + 128],
                                    es[:, :, sfx:sfx + 128],
                                    msk_sb[:, kblk:kblk + 1, :].to_broadcast((128, 2, 128)))
                            first, last = (kblk == 0), (kblk == NBLK - 1)
                            mm(a0[:, sfx:512], vf[:, 0, :], es[:, 0, sfx:512],
                               start=first, stop=last, skip_group_check=True)
                            mm(a1[:, sfx:512], vf[:, 1, :], es[:, 1, sfx:512],
                               start=first, stop=last, skip_group_check=True)
                        pair = out_pairs[hp]
                        for h, a in ((0, a0), (1, a1)):
                            srow = Pstr.tile([1, 512], F32, tag="srow")
                            nc.vector.tensor_copy(srow, a[64:65, :])
                            rec = Pstr.tile([1, 512], F32, tag="rec")
                            nc.vector.reciprocal_approx_fast(out=rec, in_=srow)
                            bc = Pstr.tile([128, 512], F32, tag="bc")
                            nc.gpsimd.partition_broadcast(bc[0:DK, :], rec)
                            nc.vector.tensor_mul(pair[h * DK:(h + 1) * DK, :],
                                                 a[0:DK, :], bc[0:DK, :])

                def ln(z_tiles, gkey, bkey, out_tiles, Pstr):
                    st0 = ps.tile([1, 512], F32, tag="u")
                    st1 = ps.tile([1, 512], F32, tag="u")
                    for k in range(NCH):
                        mm(st0, ones_t, z_tiles[k],
                           start=(k == 0), stop=(k == NCH - 1), skip_group_check=True)
                    zsq = []
                    for k in range(NCH):
                        t = Pstr.tile([128, 512], F32R, tag="zsq")
                        nc.vector.tensor_mul(t, z_tiles[k], z_tiles[k])
                        zsq.append(t)
                    for k in range(NCH):
                        mm(st1, ones_t, zsq[k],
                           start=(k == 0), stop=(k == NCH - 1), skip_group_check=True)
                    mean = Pstr.tile([1, 512], F32, tag="lnrow")
                    nc.vector.tensor_scalar_mul(mean, st0, 1.0 / D)
                    var = Pstr.tile([1, 512], F32, tag="lnrow")
                    nc.vector.tensor_scalar_mul(var, st1, 1.0 / D)
                    msq = Pstr.tile([1, 512], F32, tag="lnrow")
                    nc.vector.tensor_mul(msq, mean, mean)
                    nc.vector.tensor_sub(var, var, msq)
                    sd = Pstr.tile([1, 512], F32, tag="lnrow")
                    nc.scalar.activation(out=sd, in_=var, func=AF.Sqrt,
                                         bias=eps_t[0:1, :], scale=1.0)
                    rstd = Pstr.tile([1, 512], F32, tag="lnrow")
                    nc.vector.reciprocal_approx_fast(out=rstd, in_=sd)
                    nb = Pstr.tile([1, 512], F32, tag="lnrow")
                    nc.vector.tensor_mul(nb, mean, rstd)
                    nc.vector.tensor_scalar_mul(nb, nb, -1.0)
                    abc = Pstr.tile([128, 512], F32, tag="bc")
                    nc.gpsimd.partition_broadcast(abc, rstd)
                    bbc = Pstr.tile([128, 512], F32, tag="bc")
                    nc.gpsimd.partition_broadcast(bbc, nb)
                    for k in range(NCH):
                        t = Pstr.tile([128, 512], F32, tag="lnt")
                        nc.vector.tensor_mul(t, z_tiles[k], abc)
                        if affine:
                            t2 = Pstr.tile([128, 512], F32, tag="lnt")
                            nc.vector.tensor_add(t2, t, bbc)
                            nc.vector.tensor_scalar(out=out_tiles[k], in0=t2,
                                                    scalar1=gb_sb[gkey][:, k:k + 1],
                                                    scalar2=gb_sb[bkey][:, k:k + 1],
                                                    op0=mybir.AluOpType.mult,
                                                    op1=mybir.AluOpType.add)
                        else:
                            nc.vector.tensor_add(out_tiles[k], t, bbc)

                # deferred const loads (keep startup DMA lean)
                srcb_sb = P_const.tile([128, NBLK], F32, tag="srcb")
                nc.sync.dma_start(out=srcb_sb, in_=srcb)
                msk_sb = None
                if causal:
                    msk_sb = P_const.tile([128, NBLK, 128], BF16, tag="msk")
                    nc.sync.dma_start(out=msk_sb, in_=msk.rearrange("k p q -> p k q"))

                # =========== phase 1: self-attention + O1 + LN1 ===========
                attn_pairs = [P_y.tile([128, 512], F32R, tag=f"at{i}", name=f"atp{i}") for i in range(NPAIR)]
                y1_t = [P_y.tile([128, 512], F32R, tag=f"y{i}", name=f"y1t{i}") for i in range(NCH)]
                with tc.tile_pool(name="s1", bufs=3) as P_s1, \
                     tc.tile_pool(name="w1pool", bufs=10) as P_w1:
                    attention(qT_t, CCOUT1, causal, False, attn_pairs, P_s1)
                    wt = wload(W["o1"], P_w1)
                    for do in range(NCH):
                        p = ps.tile([128, 512], F32, tag="u")
                        for ki in range(NCH):
                            mm(p, wt[ki][:, do * 128:(do + 1) * 128],
                               attn_pairs[ki], start=(ki == 0), stop=(ki == NCH - 1))
                        o = P_s1.tile([128, 512], F32, tag="o1")
                        nc.scalar.activation(out=o, in_=p, func=AF.Identity,
                                             bias=b_sb["o1"][:, do:do + 1], scale=1.0)
                        # z1 in-place into xq tile (residual)
                        nc.vector.tensor_add(xq_t[do], o, xq_t[do])
                    ln(xq_t, "g1", "be1", y1_t, P_s1)

            # =========== phase 2: Q2 + cross-attention + O2 + LN2 ===========
            y2_t = [P_y.tile([128, 512], F32R, tag=f"y2{i}", name=f"y2t{i}") for i in range(NCH)]
            with tc.tile_pool(name="s2", bufs=3) as P_s2, \
                 tc.tile_pool(name="w2pool", bufs=10) as P_w2, \
                 tc.tile_pool(name="q2pool", bufs=1) as P_q2:
                q2_t = [P_q2.tile([128, 512], BF16, tag=f"qq{i}", name=f"q2t{i}") for i in range(NCH)]
                proj_feature(wload(W["q2"], P_w2), y1_t, b_sb["q2"], q2_t)
                attention(q2_t, CCOUT2, False, True, attn_pairs, P_s2)
                wt = wload(W["o2"], P_w2)
                for do in range(NCH):
                    p = ps.tile([128, 512], F32, tag="u")
                    for ki in range(NCH):
                        mm(p, wt[ki][:, do * 128:(do + 1) * 128],
                           attn_pairs[ki], start=(ki == 0), stop=(ki == NCH - 1))
                    o = P_s2.tile([128, 512], F32, tag="o2")
                    nc.scalar.activation(out=o, in_=p, func=AF.Identity,
                                         bias=b_sb["o2"][:, do:do + 1], scale=1.0)
                    nc.vector.tensor_add(y1_t[do], o, y1_t[do])  # z2 in-place
                ln(y1_t, "g2", "be2", y2_t, P_s2)

            # =========== phase 3: FFN + LN3 + output ===========
            with tc.tile_pool(name="s3", bufs=3) as P_s3, \
                 tc.tile_pool(name="hpool", bufs=2) as P_h, \
                 tc.tile_pool(name="wfpool", bufs=1) as P_wf, \
                 tc.tile_pool(name="holdpool", bufs=1) as P_hold:
                facc = [P_hold.tile([128, 512], F32, tag=f"fa{i}", name=f"facc{i}") for i in range(NCH)]
                y2b = []
                for i in range(NCH):
                    t = P_hold.tile([128, 512], BF16, tag=f"y2b{i}", name=f"y2b{i}")
                    nc.vector.tensor_copy(t, y2_t[i])
                    y2b.append(t)
                for g in range(8):  # groups of 4 df-chunks
                    w1g = []
                    for ki in range(NCH):
                        w = P_wf.tile([128, 512], BF16, tag="w1", name=f"w1g{ki}", bufs=12)
                        nc.gpsimd.dma_start(
                            out=w, in_=Wf1[ki * 128:(ki + 1) * 128, g * 512:(g + 1) * 512])
                        w1g.append(w)
                    hg = []
                    for j in range(4):
                        dfc = g * 4 + j
                        p = ps.tile([128, 512], F32, tag="u")
                        for ki in range(NCH):
                            mm(p, w1g[ki][:, j * 128:(j + 1) * 128], y2b[ki],
                               start=(ki == 0), stop=(ki == NCH - 1))
                        h = P_h.tile([128, 512], BF16, tag=f"h{j}")
                        nc.scalar.activation(out=h, in_=p, func=AF.Relu,
                                             bias=b_sb["f1"][:, dfc:dfc + 1], scale=1.0)
                        hg.append(h)
                    w2g = []
                    for j in range(4):
                        dfc = g * 4 + j
                        w = P_wf.tile([128, D], BF16, tag="w2", name=f"w2g{j}", bufs=6)
                        nc.gpsimd.dma_start(
                            out=w, in_=Wf2[dfc * 128:(dfc + 1) * 128, :])
                        w2g.append(w)
                    for do in range(NCH):
                        p2 = ps.tile([128, 512], F32, tag="u")
                        for j in range(4):
                            mm(p2, w2g[j][:, do * 128:(do + 1) * 128], hg[j],
                               start=(j == 0), stop=(j == 3))
                        if g == 0:
                            f = facc[do]
                            nc.vector.tensor_scalar_add(f, p2, b_sb["f2"][:, do:do + 1])
                        else:
                            nc.vector.tensor_add(facc[do], facc[do], p2)
                y3_t = [P_hold.tile([128, 512], F32, tag=f"y3{i}", name=f"y3t{i}") for i in range(NCH)]
                for do in range(NCH):
                    nc.vector.tensor_add(y2_t[do], facc[do], y2_t[do])  # z3 in-place
                ln(y2_t, "g3", "be3", y3_t, P_s3)
                for k in range(NCH):
                    nc.sync.dma_start(out=OUT[k * 128:(k + 1) * 128, :], in_=y3_t[k])

    nc.compile()
    return nc


def _get_nc(causal, affine):
    key = (causal, affine)
    if key not in _CACHE:
        _CACHE[key] = _build(causal, affine)
    return _CACHE[key]


def kernel(**inputs):
    inp = {k: np.asarray(v) for k, v in inputs.items()}
    x, enc = inp['x'].astype(np.float32), inp['enc_out'].astype(np.float32)
    tgt = np.asarray(inp['tgt_mask'])[0, 0]
    src = np.asarray(inp['src_mask'])
    causal = bool((tgt == np.tril(np.ones((S, S), tgt.dtype))).all())
    if not causal and not bool((tgt != 0).all()):
        raise NotImplementedError("tgt_mask must be causal-tril or all-ones")
    affine = not (all((inp[f'g{i}'] == 1).all() for i in (1, 2, 3))
                  and all((inp[f'be{i}'] == 0).all() for i in (1, 2, 3)))

    import ml_dtypes
    BF = ml_dtypes.bfloat16
    WT = {k: np.ascontiguousarray(inp['W' + k].T) for k in
          ['q1', 'k1', 'v1', 'o1', 'q2', 'k2', 'v2', 'o2']}
    WT['f1'] = np.ascontiguousarray(inp['Wf1'].T.astype(BF))
    WT['f2'] = np.ascontiguousarray(inp['Wf2'].T.astype(BF))
    bch = {k: np.ascontiguousarray(inp['b' + k].reshape(-1, 128).T)
           for k in ['q1', 'k1', 'o1', 'q2', 'k2', 'o2', 'f1', 'f2']}

    nc = _get_nc(causal, affine)

    in_maps = []
    for c in range(8):
        b, q = c // 4, c % 4
        qblocks = [q + 4 * j for j in range(4)]
        qrows = np.concatenate([np.arange(g * 128, g * 128 + 128) for g in qblocks])
        m = {
            'xqT': np.ascontiguousarray(x[b, qrows].T),
            'xkT': np.ascontiguousarray(x[b, q * 512:(q + 1) * 512].T),
            'encT': np.ascontiguousarray(enc[b, q * 512:(q + 1) * 512].T),
            'bv1': np.ascontiguousarray(inp['bv1'][None, :]),
            'bv2': np.ascontiguousarray(inp['bv2'][None, :]),
            'ones_in': np.ones((128, H, 1), ml_dtypes.bfloat16),
            'ones_f': np.ones((128, 1), np.float32),
            'srcb': np.ascontiguousarray(
                np.where(src[b, 0, 0] == 0, np.float32(-1e9), np.float32(0.0))
                .astype(np.float32).reshape(NBLK, 128).T),
        }
        for k in ['q1', 'k1', 'v1', 'o1', 'q2', 'k2', 'v2', 'o2']:
            m['W' + k] = WT[k]
        m['Wf1T'] = WT['f1']
        m['Wf2T'] = WT['f2']
        for k in ['q1', 'k1', 'o1', 'q2', 'k2', 'o2', 'f2', 'f1']:
            m['b' + k] = bch[k]
        if causal:
            ms = np.empty((NBLK, 128, 128), np.float32)
            for kblk in range(NBLK):
                gq = qblocks[kblk // 4]
                ms[kblk] = tgt[gq * 128:(gq + 1) * 128,
                               kblk * 128:(kblk + 1) * 128].T.astype(np.float32)
            m['mself'] = np.ascontiguousarray(ms.astype(ml_dtypes.bfloat16))
        if affine:
            for k in ['g1', 'be1', 'g2', 'be2', 'g3', 'be3']:
                m[k] = np.ascontiguousarray(inp[k].reshape(NCH, 128).T)
        in_maps.append(m)

    trace = bool(int(os.environ.get("KERNEL_TRACE", "0")))
    res = bass_utils.run_bass_kernel_spmd(
        nc, in_maps, core_ids=list(range(8)), trace=trace,
        tmpdir=(tempfile.mkdtemp(prefix="declayer_") if trace else None))
    kernel._last_results = res

    out = np.zeros((B, S, D), np.float32)
    for c in range(8):
        b, q = c // 4, c % 4
        qblocks = [q + 4 * j for j in range(4)]
        qrows = np.concatenate([np.arange(g * 128, g * 128 + 128) for g in qblocks])
        out[b, qrows] = res.results[c]['OUT'].T
    return out


# revision 29
# speedup vs baseline: 1.7934x; 1.0751x over previous
"""Trainium2 Bass kernel for nn_DecoderLayer (self-attn + cross-attn + FFN, 3 LNs).

Sharding: 8 cores = 2 batches x 4 query-shards. Core c handles batch c//4 and
query blocks {q, q+4, q+8, q+12} (q = c%4, blocks of 128 rows) — stride-4 for
causal load balance with a padded-uniform suffix structure so all cores run the
same SPMD program. K/V projections are computed on contiguous 512-row shards
and exchanged with a single AllGather (self KV + cross KV together).

Layouts: activations feature-major (x.T: [d, seq] with d on partitions);
V position-major ([seq, dv]) so attn@V needs no transposes; scores computed
transposed ([kpos, q]) with softmax sums taken via an appended ones-column in
the V matmul. All matmuls run in float32r (full PE rate, ~1.5e-4 rel err).
"""
import os
import sys
import tempfile

import numpy as np

sys.path.insert(0, '/opt/trn_rl_repo')

import concourse.mybir as mybir  # noqa: E402
import concourse.tile as tile  # noqa: E402
from concourse import bacc, bass_utils  # noqa: E402

B, S, T, D, H, DK, DF = 2, 2048, 2048, 1024, 16, 64, 4096
EPS = 1e-5
NBLK = S // 128          # 16 k-blocks
NCH = D // 128           # 8 feature chunks
NPAIR = H // 2           # 8 head pairs
VW = DK + 1              # V row width per head (ones column baked in)
KSEG = 1024 * 512
VSEG = 512 * H * VW
SEG = KSEG + VSEG        # per-rank elements of ONE AllGather (self or cross)

F32 = mybir.dt.float32
F32R = mybir.dt.float32r
BF16 = mybir.dt.bfloat16
AF = mybir.ActivationFunctionType

_CACHE = {}


def _R(ap):
    return ap.bitcast(F32R) if ap.dtype == F32 else ap


def _sfx(kblk, causal):
    return 128 * (kblk // 4) if causal else 0


def _build(causal, affine):
    nc = bacc.Bacc("TRN2", target_bir_lowering=False, debug=False, num_devices=8)

    def mm(out, lhsT, rhs, **kw):
        nc.tensor.matmul(out, _R(lhsT), _R(rhs), **kw)

    def din(name, shape, dtype=F32):
        return nc.dram_tensor(name, shape, dtype, kind="ExternalInput").ap()

    xqT = din("xqT", [D, 512], F32R)
    xkT = din("xkT", [D, 512], F32R)
    encT = din("encT", [D, 512], F32R)
    W = {k: din("W" + k, [D, D], F32R) for k in ["q1", "k1", "v1", "o1", "q2", "k2", "v2", "o2"]}
    Wf1 = din("Wf1T", [D, DF], BF16)
    Wf2 = din("Wf2T", [DF, D], BF16)
    bias_in = {k: din("b" + k, [128, NCH]) for k in ["q1", "k1", "o1", "q2", "k2", "o2", "f2"]}
    bias_in["f1"] = din("bf1", [128, DF // 128])
    bv1 = din("bv1", [1, D])
    bv2 = din("bv2", [1, D])
    srcb = din("srcb", [128, NBLK])
    ones_in = din("ones_in", [128, H, 1], BF16)
    ones_f = din("ones_f", [128, 1], F32R)
    if causal:
        msk = din("mself", [NBLK, 128, 128], BF16)
    gb = {}
    if affine:
        for k in ["g1", "be1", "g2", "be2", "g3", "be3"]:
            gb[k] = din(k, [128, NCH])
    OUT = nc.dram_tensor("OUT", [D, 512], F32, kind="ExternalOutput").ap()

    CCIN1 = nc.dram_tensor("ccin1", [SEG], BF16).ap()
    CCOUT1 = nc.dram_tensor("ccout1", [4 * SEG], BF16).ap()
    CCIN2 = nc.dram_tensor("ccin2", [SEG], BF16).ap()
    CCOUT2 = nc.dram_tensor("ccout2", [4 * SEG], BF16).ap()

    with tile.TileContext(nc) as tc:
        with tc.tile_pool(name="const", bufs=1) as P_const, \
             tc.tile_pool(name="ps", bufs=3, space="PSUM") as ps, \
             tc.tile_pool(name="psatt", bufs=2, space="PSUM") as ps_att, \
             tc.tile_pool(name="ypool", bufs=1) as P_y:

            # ---- constants ----
            ones_t = P_const.tile([128, 1], F32R, tag="ones")
            nc.sync.dma_start(out=ones_t, in_=ones_f)
            eps_t = P_const.tile([128, 1], F32, tag="eps")
            nc.vector.memset(eps_t, EPS)
            b_sb = {}
            for k, ap_ in bias_in.items():
                t = P_const.tile(list(ap_.shape), F32, tag="b" + k)
                nc.sync.dma_start(out=t, in_=ap_)
                b_sb[k] = t
            gb_sb = {}
            if affine:
                for k in gb:
                    t = P_const.tile([128, NCH], F32, tag=k)
                    nc.sync.dma_start(out=t, in_=gb[k])
                    gb_sb[k] = t

            def wload(Wap, pool):
                tiles = []
                for ki in range(NCH):
                    t = pool.tile([128, Wap.shape[1]], F32R, tag="w", name=f"wt{ki}")
                    nc.gpsimd.dma_start(out=t, in_=Wap[ki * 128:(ki + 1) * 128, :])
                    tiles.append(t)
                return tiles

            def proj_feature(wt, rhs_tiles, bias_t, out_tiles, act=AF.Identity):
                for do in range(NCH):
                    p = ps.tile([128, 512], F32, tag="u")
                    for ki in range(NCH):
                        mm(p, wt[ki][:, do * 128:(do + 1) * 128],
                           rhs_tiles[ki], start=(ki == 0), stop=(ki == NCH - 1))
                    nc.scalar.activation(out=out_tiles[do], in_=p, func=act,
                                         bias=bias_t[:, do:do + 1], scale=1.0)

            # =========== phase 0: KV projections + AllGather + Q ===========
            with tc.tile_pool(name="xqpool", bufs=1) as P_xq:
                xq_t = []
                qT_t = []
                with tc.tile_pool(name="p0", bufs=1) as P0, \
                     tc.tile_pool(name="p0w", bufs=10) as P_w0, \
                     tc.tile_pool(name="p0s", bufs=3) as P0s:
                    onesbc = P0.tile([128, H, 1], BF16, tag="onesbc")
                    nc.sync.dma_start(out=onesbc, in_=ones_in)
                    xk_t, enc_t = [], []
                    for ki in range(NCH):
                        t = P0.tile([128, 512], F32R, tag=f"xk{ki}")
                        nc.sync.dma_start(out=t, in_=xkT[ki * 128:(ki + 1) * 128, :])
                        xk_t.append(t)
                    for ki in range(NCH):
                        t = P0.tile([128, 512], F32R, tag=f"en{ki}")
                        nc.sync.dma_start(out=t, in_=encT[ki * 128:(ki + 1) * 128, :])
                        enc_t.append(t)
                    bvbc1 = P0.tile([128, D], F32, tag="bvbc1")
                    r1 = P0.tile([1, D], F32, tag="bvr1")
                    nc.sync.dma_start(out=r1, in_=bv1)
                    nc.gpsimd.partition_broadcast(bvbc1, r1)
                    bvbc2 = P0.tile([128, D], F32, tag="bvbc2")
                    r2 = P0.tile([1, D], F32, tag="bvr2")
                    nc.sync.dma_start(out=r2, in_=bv2)
                    nc.gpsimd.partition_broadcast(bvbc2, r2)

                    def kproj(wkey, rhs, bkey, ccin):
                        wt = wload(W[wkey], P_w0)
                        for do in range(NCH):
                            p = ps.tile([128, 512], F32, tag="u")
                            for ki in range(NCH):
                                mm(p, wt[ki][:, do * 128:(do + 1) * 128],
                                   rhs[ki], start=(ki == 0), stop=(ki == NCH - 1))
                            o = P0s.tile([128, 512], BF16, tag="kvo")
                            nc.scalar.activation(out=o, in_=p, func=AF.Identity,
                                                 bias=b_sb[bkey][:, do:do + 1], scale=1.0)
                            dst = ccin[do * 128 * 512:(do + 1) * 128 * 512]
                            nc.sync.dma_start(out=dst.rearrange("(p s) -> p s", s=512), in_=o)

                    def vproj(wkey, lhs, bvbc, ccin):
                        wt = wload(W[wkey], P_w0)
                        for sc in range(4):
                            p = ps.tile([128, D], F32, tag="u")
                            for ki in range(NCH):
                                for half in range(2):
                                    mm(p[:, half * 512:(half + 1) * 512],
                                       lhs[ki][:, sc * 128:(sc + 1) * 128],
                                       wt[ki][:, half * 512:(half + 1) * 512],
                                       start=(ki == 0), stop=(ki == NCH - 1))
                            o = P0s.tile([128, H, VW], BF16, tag="kvo2")
                            nc.vector.tensor_add(
                                o[:, :, 0:DK],
                                p.rearrange("p (h v) -> p h v", v=DK),
                                bvbc.rearrange("p (h v) -> p h v", v=DK))
                            nc.vector.tensor_copy(o[:, :, DK:VW], onesbc)
                            dst = ccin[KSEG + sc * 128 * H * VW:
                                       KSEG + (sc + 1) * 128 * H * VW]
                            nc.sync.dma_start(
                                out=dst.rearrange("(p h v) -> p h v", h=H, v=VW), in_=o)

                    kproj("k1", xk_t, "k1", CCIN1)
                    vproj("v1", xk_t, bvbc1, CCIN1)
                    nc.gpsimd.collective_compute(
                        "AllGather", mybir.AluOpType.bypass,
                        ins=[CCIN1], outs=[CCOUT1],
                        replica_groups=[[0, 1, 2, 3], [4, 5, 6, 7]],
                    )
                    kproj("k2", enc_t, "k2", CCIN2)
                    vproj("v2", enc_t, bvbc2, CCIN2)
                    nc.gpsimd.collective_compute(
                        "AllGather", mybir.AluOpType.bypass,
                        ins=[CCIN2], outs=[CCOUT2],
                        replica_groups=[[0, 1, 2, 3], [4, 5, 6, 7]],
                    )

                    # Q projection (overlaps the AllGathers)
                    for ki in range(NCH):
                        t = P_xq.tile([128, 512], F32R, tag=f"xq{ki}")
                        nc.sync.dma_start(out=t, in_=xqT[ki * 128:(ki + 1) * 128, :])
                        xq_t.append(t)
                    qT_t = [P_xq.tile([128, 512], BF16, tag=f"q{i}", name=f"qT{i}") for i in range(NCH)]
                    proj_feature(wload(W["q1"], P_w0), xq_t, b_sb["q1"], qT_t)

                # ---- shared attention ----
                def attention(qtiles, ccout, causal_, use_srcb, out_pairs, Pstr):
                    # V resident: one [128, H, VW] tile per k-block, shared by all pairs
                    vres = []
                    for kblk in range(NBLK):
                        r, lb = kblk // 4, kblk % 4
                        vt = Pstr.tile([128, H, VW], BF16, tag=f"vres{kblk}", bufs=1,
                                       name=f"vres{kblk}")
                        vsrc = ccout[r * SEG + KSEG + lb * 128 * H * VW:
                                     r * SEG + KSEG + (lb + 1) * 128 * H * VW]
                        nc.sync.dma_start(
                            out=vt, in_=vsrc.rearrange("(p h v) -> p h v", h=H, v=VW))
                        vres.append(vt)
                    for hp in range(NPAIR):
                        kt = Pstr.tile([128, 4, 512], BF16, tag="kt")
                        for r in range(4):
                            src = ccout[r * SEG + hp * 128 * 512:
                                        r * SEG + (hp + 1) * 128 * 512]
                            nc.sync.dma_start(out=kt[:, r, :],
                                               in_=src.rearrange("(p s) -> p s", s=512))
                        a0 = ps_att.tile([65, 512], F32, tag="a")
                        a1 = ps_att.tile([65, 512], F32, tag="a")
                        for kblk in range(NBLK):
                            sfx = _sfx(kblk, causal_)
                            if sfx >= 512:
                                continue
                            r, lb = kblk // 4, kblk % 4
                            vf = vres[kblk]

                            sc_ps = ps.tile([128, 2, 512], F32, tag="u")
                            for h in range(2):
                                bp = h * DK
                                mm(sc_ps[:, h, sfx:512],
                                   kt[bp:bp + DK, r, lb * 128:lb * 128 + 128],
                                   qtiles[hp][bp:bp + DK, sfx:512],
                                   start=True, stop=True, tile_position=(bp, 0))
                            es = Pstr.tile([128, 2, 512], BF16, tag="es")
                            if use_srcb:
                                nc.scalar.activation(out=es[:, :, sfx:512],
                                                     in_=sc_ps[:, :, sfx:512],
                                                     func=AF.Exp, scale=0.125,
                                                     bias=srcb_sb[:, kblk:kblk + 1])
                            else:
                                nc.scalar.activation(out=es[:, :, sfx:512],
                                                     in_=sc_ps[:, :, sfx:512],
                                                     func=AF.Exp, scale=0.125)
                            if causal_:
                                nc.vector.tensor_mul(
                                    es[:, :, sfx:sfx + 128],
                                    es[:, :, sfx:sfx + 128],
                                    msk_sb[:, kblk:kblk + 1, :].to_broadcast((128, 2, 128)))
                            first, last = (kblk == 0), (kblk == NBLK - 1)
                            mm(a0[:, sfx:512], vf[:, 2 * hp, :], es[:, 0, sfx:512],
                               start=first, stop=last, skip_group_check=True)
                            mm(a1[:, sfx:512], vf[:, 2 * hp + 1, :], es[:, 1, sfx:512],
                               start=first, stop=last, skip_group_check=True)
                        pair = out_pairs[hp]
                        for h, a in ((0, a0), (1, a1)):
                            srow = Pstr.tile([1, 512], F32, tag="srow")
                            nc.vector.tensor_copy(srow, a[64:65, :])
                            rec = Pstr.tile([1, 512], F32, tag="rec")
                            nc.vector.reciprocal_approx_fast(out=rec, in_=srow)
                            bc = Pstr.tile([128, 512], F32, tag="bc")
                            nc.gpsimd.partition_broadcast(bc[0:DK, :], rec)
                            nc.vector.tensor_mul(pair[h * DK:(h + 1) * DK, :],
                                                 a[0:DK, :], bc[0:DK, :])

                def ln(z_tiles, gkey, bkey, out_tiles, Pstr):
                    st0 = ps.tile([1, 512], F32, tag="u")
                    st1 = ps.tile([1, 512], F32, tag="u")
                    for k in range(NCH):
                        mm(st0, ones_t, z_tiles[k],
                           start=(k == 0), stop=(k == NCH - 1), skip_group_check=True)
                    zsq = []
                    for k in range(NCH):
                        t = Pstr.tile([128, 512], F32R, tag="zsq")
                        nc.vector.tensor_mul(t, z_tiles[k], z_tiles[k])
                        zsq.append(t)
                    for k in range(NCH):
                        mm(st1, ones_t, zsq[k],
                           start=(k == 0), stop=(k == NCH - 1), skip_group_check=True)
                    mean = Pstr.tile([1, 512], F32, tag="lnrow")
                    nc.vector.tensor_scalar_mul(mean, st0, 1.0 / D)
                    var = Pstr.tile([1, 512], F32, tag="lnrow")
                    nc.vector.tensor_scalar_mul(var, st1, 1.0 / D)
                    msq = Pstr.tile([1, 512], F32, tag="lnrow")
                    nc.vector.tensor_mul(msq, mean, mean)
                    nc.vector.tensor_sub(var, var, msq)
                    sd = Pstr.tile([1, 512], F32, tag="lnrow")
                    nc.scalar.activation(out=sd, in_=var, func=AF.Sqrt,
                                         bias=eps_t[0:1, :], scale=1.0)
                    rstd = Pstr.tile([1, 512], F32, tag="lnrow")
                    nc.vector.reciprocal_approx_fast(out=rstd, in_=sd)
                    nb = Pstr.tile([1, 512], F32, tag="lnrow")
                    nc.vector.tensor_mul(nb, mean, rstd)
                    nc.vector.tensor_scalar_mul(nb, nb, -1.0)
                    abc = Pstr.tile([128, 512], F32, tag="bc")
                    nc.gpsimd.partition_broadcast(abc, rstd)
                    bbc = Pstr.tile([128, 512], F32, tag="bc")
                    nc.gpsimd.partition_broadcast(bbc, nb)
                    for k in range(NCH):
                        t = Pstr.tile([128, 512], F32, tag="lnt")
                        nc.vector.tensor_mul(t, z_tiles[k], abc)
                        if affine:
                            t2 = Pstr.tile([128, 512], F32, tag="lnt")
                            nc.vector.tensor_add(t2, t, bbc)
                            nc.vector.tensor_scalar(out=out_tiles[k], in0=t2,
                                                    scalar1=gb_sb[gkey][:, k:k + 1],
                                                    scalar2=gb_sb[bkey][:, k:k + 1],
                                                    op0=mybir.AluOpType.mult,
                                                    op1=mybir.AluOpType.add)
                        else:
                            nc.vector.tensor_add(out_tiles[k], t, bbc)

                # deferred const loads (keep startup DMA lean)
                srcb_sb = P_const.tile([128, NBLK], F32, tag="srcb")
                nc.sync.dma_start(out=srcb_sb, in_=srcb)
                msk_sb = None
                if causal:
                    msk_sb = P_const.tile([128, NBLK, 128], BF16, tag="msk")
                    nc.sync.dma_start(out=msk_sb, in_=msk.rearrange("k p q -> p k q"))

                # =========== phase 1: self-attention + O1 + LN1 ===========
                attn_pairs = [P_y.tile([128, 512], F32R, tag=f"at{i}", name=f"atp{i}") for i in range(NPAIR)]
                y1_t = [P_y.tile([128, 512], F32R, tag=f"y{i}", name=f"y1t{i}") for i in range(NCH)]
                with tc.tile_pool(name="s1", bufs=3) as P_s1, \
                     tc.tile_pool(name="w1pool", bufs=9) as P_w1:
                    attention(qT_t, CCOUT1, causal, False, attn_pairs, P_s1)
                    wt = wload(W["o1"], P_w1)
                    for do in range(NCH):
                        p = ps.tile([128, 512], F32, tag="u")
                        for ki in range(NCH):
                            mm(p, wt[ki][:, do * 128:(do + 1) * 128],
                               attn_pairs[ki], start=(ki == 0), stop=(ki == NCH - 1))
                        o = P_s1.tile([128, 512], F32, tag="o1")
                        nc.scalar.activation(out=o, in_=p, func=AF.Identity,
                                             bias=b_sb["o1"][:, do:do + 1], scale=1.0)
                        # z1 in-place into xq tile (residual)
                        nc.vector.tensor_add(xq_t[do], o, xq_t[do])
                    ln(xq_t, "g1", "be1", y1_t, P_s1)

            # =========== phase 2: Q2 + cross-attention + O2 + LN2 ===========
            y2_t = [P_y.tile([128, 512], F32R, tag=f"y2{i}", name=f"y2t{i}") for i in range(NCH)]
            with tc.tile_pool(name="s2", bufs=3) as P_s2, \
                 tc.tile_pool(name="w2pool", bufs=9) as P_w2, \
                 tc.tile_pool(name="q2pool", bufs=1) as P_q2:
                q2_t = [P_q2.tile([128, 512], BF16, tag=f"qq{i}", name=f"q2t{i}") for i in range(NCH)]
                proj_feature(wload(W["q2"], P_w2), y1_t, b_sb["q2"], q2_t)
                attention(q2_t, CCOUT2, False, True, attn_pairs, P_s2)
                wt = wload(W["o2"], P_w2)
                for do in range(NCH):
                    p = ps.tile([128, 512], F32, tag="u")
                    for ki in range(NCH):
                        mm(p, wt[ki][:, do * 128:(do + 1) * 128],
                           attn_pairs[ki], start=(ki == 0), stop=(ki == NCH - 1))
                    o = P_s2.tile([128, 512], F32, tag="o2")
                    nc.scalar.activation(out=o, in_=p, func=AF.Identity,
                                         bias=b_sb["o2"][:, do:do + 1], scale=1.0)
                    nc.vector.tensor_add(y1_t[do], o, y1_t[do])  # z2 in-place
                ln(y1_t, "g2", "be2", y2_t, P_s2)

            # =========== phase 3: FFN + LN3 + output ===========
            with tc.tile_pool(name="s3", bufs=3) as P_s3, \
                 tc.tile_pool(name="hpool", bufs=2) as P_h, \
                 tc.tile_pool(name="wfpool", bufs=1) as P_wf, \
                 tc.tile_pool(name="holdpool", bufs=1) as P_hold:
                facc = [P_hold.tile([128, 512], F32, tag=f"fa{i}", name=f"facc{i}") for i in range(NCH)]
                y2b = []
                for i in range(NCH):
                    t = P_hold.tile([128, 512], BF16, tag=f"y2b{i}", name=f"y2b{i}")
                    nc.vector.tensor_copy(t, y2_t[i])
                    y2b.append(t)
                for g in range(8):  # groups of 4 df-chunks
                    w1g = []
                    for ki in range(NCH):
                        w = P_wf.tile([128, 512], BF16, tag="w1", name=f"w1g{ki}", bufs=12)
                        nc.gpsimd.dma_start(
                            out=w, in_=Wf1[ki * 128:(ki + 1) * 128, g * 512:(g + 1) * 512])
                        w1g.append(w)
                    hg = []
                    for j in range(4):
                        dfc = g * 4 + j
                        p = ps.tile([128, 512], F32, tag="u")
                        for ki in range(NCH):
                            mm(p, w1g[ki][:, j * 128:(j + 1) * 128], y2b[ki],
                               start=(ki == 0), stop=(ki == NCH - 1))
                        h = P_h.tile([128, 512], BF16, tag=f"h{j}")
                        nc.scalar.activation(out=h, in_=p, func=AF.Relu,
                                             bias=b_sb["f1"][:, dfc:dfc + 1], scale=1.0)
                        hg.append(h)
                    w2g = []
                    for j in range(4):
                        dfc = g * 4 + j
                        w = P_wf.tile([128, D], BF16, tag="w2", name=f"w2g{j}", bufs=6)
                        nc.gpsimd.dma_start(
                            out=w, in_=Wf2[dfc * 128:(dfc + 1) * 128, :])
                        w2g.append(w)
                    for do in range(NCH):
                        p2 = ps.tile([128, 512], F32, tag="u")
                        for j in range(4):
                            mm(p2, w2g[j][:, do * 128:(do + 1) * 128], hg[j],
                               start=(j == 0), stop=(j == 3))
                        if g == 0:
                            f = facc[do]
                            nc.vector.tensor_scalar_add(f, p2, b_sb["f2"][:, do:do + 1])
                        else:
                            nc.vector.tensor_add(facc[do], facc[do], p2)
                y3_t = [P_hold.tile([128, 512], F32, tag=f"y3{i}", name=f"y3t{i}") for i in range(NCH)]
                for do in range(NCH):
                    nc.vector.tensor_add(y2_t[do], facc[do], y2_t[do])  # z3 in-place
                ln(y2_t, "g3", "be3", y3_t, P_s3)
                for k in range(NCH):
                    nc.sync.dma_start(out=OUT[k * 128:(k + 1) * 128, :], in_=y3_t[k])

    nc.compile()
    return nc


def _get_nc(causal, affine):
    key = (causal, affine)
    if key not in _CACHE:
        _CACHE[key] = _build(causal, affine)
    return _CACHE[key]


def kernel(**inputs):
    inp = {k: np.asarray(v) for k, v in inputs.items()}
    x, enc = inp['x'].astype(np.float32), inp['enc_out'].astype(np.float32)
    tgt = np.asarray(inp['tgt_mask'])[0, 0]
    src = np.asarray(inp['src_mask'])
    causal = bool((tgt == np.tril(np.ones((S, S), tgt.dtype))).all())
    if not causal and not bool((tgt != 0).all()):
        raise NotImplementedError("tgt_mask must be causal-tril or all-ones")
    affine = not (all((inp[f'g{i}'] == 1).all() for i in (1, 2, 3))
                  and all((inp[f'be{i}'] == 0).all() for i in (1, 2, 3)))

    import ml_dtypes
    BF = ml_dtypes.bfloat16
    WT = {k: np.ascontiguousarray(inp['W' + k].T) for k in
          ['q1', 'k1', 'v1', 'o1', 'q2', 'k2', 'v2', 'o2']}
    WT['f1'] = np.ascontiguousarray(inp['Wf1'].T.astype(BF))
    WT['f2'] = np.ascontiguousarray(inp['Wf2'].T.astype(BF))
    bch = {k: np.ascontiguousarray(inp['b' + k].reshape(-1, 128).T)
           for k in ['q1', 'k1', 'o1', 'q2', 'k2', 'o2', 'f1', 'f2']}

    nc = _get_nc(causal, affine)

    in_maps = []
    for c in range(8):
        b, q = c // 4, c % 4
        qblocks = [q + 4 * j for j in range(4)]
        qrows = np.concatenate([np.arange(g * 128, g * 128 + 128) for g in qblocks])
        m = {
            'xqT': np.ascontiguousarray(x[b, qrows].T),
            'xkT': np.ascontiguousarray(x[b, q * 512:(q + 1) * 512].T),
            'encT': np.ascontiguousarray(enc[b, q * 512:(q + 1) * 512].T),
            'bv1': np.ascontiguousarray(inp['bv1'][None, :]),
            'bv2': np.ascontiguousarray(inp['bv2'][None, :]),
            'ones_in': np.ones((128, H, 1), ml_dtypes.bfloat16),
            'ones_f': np.ones((128, 1), np.float32),
            'srcb': np.ascontiguousarray(
                np.where(src[b, 0, 0] == 0, np.float32(-1e9), np.float32(0.0))
                .astype(np.float32).reshape(NBLK, 128).T),
        }
        for k in ['q1', 'k1', 'v1', 'o1', 'q2', 'k2', 'v2', 'o2']:
            m['W' + k] = WT[k]
        m['Wf1T'] = WT['f1']
        m['Wf2T'] = WT['f2']
        for k in ['q1', 'k1', 'o1', 'q2', 'k2', 'o2', 'f2', 'f1']:
            m['b' + k] = bch[k]
        if causal:
            ms = np.empty((NBLK, 128, 128), np.float32)
            for kblk in range(NBLK):
                gq = qblocks[kblk // 4]
                ms[kblk] = tgt[gq * 128:(gq + 1) * 128,
                               kblk * 128:(kblk + 1) * 128].T.astype(np.float32)
            m['mself'] = np.ascontiguousarray(ms.astype(ml_dtypes.bfloat16))
        if affine:
            for k in ['g1', 'be1', 'g2', 'be2', 'g3', 'be3']:
                m[k] = np.ascontiguousarray(inp[k].reshape(NCH, 128).T)
        in_maps.append(m)

    trace = bool(int(os.environ.get("KERNEL_TRACE", "0")))
    res = bass_utils.run_bass_kernel_spmd(
        nc, in_maps, core_ids=list(range(8)), trace=trace,
        tmpdir=(tempfile.mkdtemp(prefix="declayer_") if trace else None))
    kernel._last_results = res

    out = np.zeros((B, S, D), np.float32)
    for c in range(8):
        b, q = c // 4, c % 4
        qblocks = [q + 4 * j for j in range(4)]
        qrows = np.concatenate([np.arange(g * 128, g * 128 + 128) for g in qblocks])
        out[b, qrows] = res.results[c]['OUT'].T
    return out


# revision 30
# speedup vs baseline: 1.8671x; 1.0411x over previous
"""Trainium2 Bass kernel for nn_DecoderLayer (self-attn + cross-attn + FFN, 3 LNs).

Sharding: 8 cores = 2 batches x 4 query-shards. Core c handles batch c//4 and
query blocks {q, q+4, q+8, q+12} (q = c%4, blocks of 128 rows) — stride-4 for
causal load balance with a padded-uniform suffix structure so all cores run the
same SPMD program. K/V projections are computed on contiguous 512-row shards
and exchanged with a single AllGather (self KV + cross KV together).

Layouts: activations feature-major (x.T: [d, seq] with d on partitions);
V position-major ([seq, dv]) so attn@V needs no transposes; scores computed
transposed ([kpos, q]) with softmax sums taken via an appended ones-column in
the V matmul. All matmuls run in float32r (full PE rate, ~1.5e-4 rel err).
"""
import os
import sys
import tempfile

import numpy as np

sys.path.insert(0, '/opt/trn_rl_repo')

import concourse.mybir as mybir  # noqa: E402
import concourse.tile as tile  # noqa: E402
from concourse import bacc, bass_utils  # noqa: E402

B, S, T, D, H, DK, DF = 2, 2048, 2048, 1024, 16, 64, 4096
EPS = 1e-5
NBLK = S // 128          # 16 k-blocks
NCH = D // 128           # 8 feature chunks
NPAIR = H // 2           # 8 head pairs
VW = DK + 1              # V row width per head (ones column baked in)
KSEG = 1024 * 512
VSEG = 512 * H * VW
SEG = KSEG + VSEG        # per-rank elements of ONE AllGather (self or cross)

F32 = mybir.dt.float32
F32R = mybir.dt.float32r
BF16 = mybir.dt.bfloat16
AF = mybir.ActivationFunctionType

_CACHE = {}


def _R(ap):
    return ap.bitcast(F32R) if ap.dtype == F32 else ap


def _sfx(kblk, causal):
    return 128 * (kblk // 4) if causal else 0


def _build(causal, affine):
    nc = bacc.Bacc("TRN2", target_bir_lowering=False, debug=False, num_devices=8)

    def mm(out, lhsT, rhs, **kw):
        nc.tensor.matmul(out, _R(lhsT), _R(rhs), **kw)

    def din(name, shape, dtype=F32):
        return nc.dram_tensor(name, shape, dtype, kind="ExternalInput").ap()

    xqT = din("xqT", [D, 512], F32R)
    xkT = din("xkT", [D, 512], BF16)
    encT = din("encT", [D, 512], BF16)
    W = {"q1": din("Wq1", [D, D], F32R)}
    for k in ["k1", "v1", "o1", "q2", "k2", "v2", "o2"]:
        W[k] = din("W" + k, [D, D], BF16)
    Wf1 = din("Wf1T", [D, DF], BF16)
    Wf2 = din("Wf2T", [DF, D], BF16)
    bias_in = {k: din("b" + k, [128, NCH]) for k in ["q1", "k1", "o1", "q2", "k2", "o2", "f2"]}
    bias_in["f1"] = din("bf1", [128, DF // 128])
    bv1 = din("bv1", [1, D])
    bv2 = din("bv2", [1, D])
    srcb = din("srcb", [128, NBLK])
    ones_in = din("ones_in", [128, H, 1], BF16)
    ones_f = din("ones_f", [128, 1], F32R)
    if causal:
        msk = din("mself", [NBLK, 128, 128], BF16)
    gb = {}
    if affine:
        for k in ["g1", "be1", "g2", "be2", "g3", "be3"]:
            gb[k] = din(k, [128, NCH])
    OUT = nc.dram_tensor("OUT", [D, 512], F32, kind="ExternalOutput").ap()

    CCIN1 = nc.dram_tensor("ccin1", [SEG], BF16).ap()
    CCOUT1 = nc.dram_tensor("ccout1", [4 * SEG], BF16).ap()
    CCIN2 = nc.dram_tensor("ccin2", [SEG], BF16).ap()
    CCOUT2 = nc.dram_tensor("ccout2", [4 * SEG], BF16).ap()

    with tile.TileContext(nc) as tc:
        with tc.tile_pool(name="const", bufs=1) as P_const, \
             tc.tile_pool(name="ps", bufs=3, space="PSUM") as ps, \
             tc.tile_pool(name="psatt", bufs=2, space="PSUM") as ps_att, \
             tc.tile_pool(name="ypool", bufs=1) as P_y:

            # ---- constants ----
            ones_t = P_const.tile([128, 1], F32R, tag="ones")
            nc.sync.dma_start(out=ones_t, in_=ones_f)
            eps_t = P_const.tile([128, 1], F32, tag="eps")
            nc.vector.memset(eps_t, EPS)
            b_sb = {}
            for k, ap_ in bias_in.items():
                t = P_const.tile(list(ap_.shape), F32, tag="b" + k)
                nc.sync.dma_start(out=t, in_=ap_)
                b_sb[k] = t
            gb_sb = {}
            if affine:
                for k in gb:
                    t = P_const.tile([128, NCH], F32, tag=k)
                    nc.sync.dma_start(out=t, in_=gb[k])
                    gb_sb[k] = t

            def wload(Wap, pool):
                tiles = []
                for ki in range(NCH):
                    t = pool.tile([128, Wap.shape[1]], Wap.dtype, tag="w", name=f"wt{ki}")
                    nc.gpsimd.dma_start(out=t, in_=Wap[ki * 128:(ki + 1) * 128, :])
                    tiles.append(t)
                return tiles

            def proj_feature(wt, rhs_tiles, bias_t, out_tiles, act=AF.Identity):
                for do in range(NCH):
                    p = ps.tile([128, 512], F32, tag="u")
                    for ki in range(NCH):
                        mm(p, wt[ki][:, do * 128:(do + 1) * 128],
                           rhs_tiles[ki], start=(ki == 0), stop=(ki == NCH - 1))
                    nc.scalar.activation(out=out_tiles[do], in_=p, func=act,
                                         bias=bias_t[:, do:do + 1], scale=1.0)

            # =========== phase 0: KV projections + AllGather + Q ===========
            with tc.tile_pool(name="xqpool", bufs=1) as P_xq:
                xq_t = []
                qT_t = []
                with tc.tile_pool(name="p0", bufs=1) as P0, \
                     tc.tile_pool(name="p0w", bufs=10) as P_w0, \
                     tc.tile_pool(name="p0s", bufs=3) as P0s:
                    onesbc = P0.tile([128, H, 1], BF16, tag="onesbc")
                    nc.sync.dma_start(out=onesbc, in_=ones_in)
                    xk_t, enc_t = [], []
                    for ki in range(NCH):
                        t = P0.tile([128, 512], BF16, tag=f"xk{ki}")
                        nc.sync.dma_start(out=t, in_=xkT[ki * 128:(ki + 1) * 128, :])
                        xk_t.append(t)
                    for ki in range(NCH):
                        t = P0.tile([128, 512], BF16, tag=f"en{ki}")
                        nc.sync.dma_start(out=t, in_=encT[ki * 128:(ki + 1) * 128, :])
                        enc_t.append(t)
                    bvbc1 = P0.tile([128, D], F32, tag="bvbc1")
                    r1 = P0.tile([1, D], F32, tag="bvr1")
                    nc.sync.dma_start(out=r1, in_=bv1)
                    nc.gpsimd.partition_broadcast(bvbc1, r1)
                    bvbc2 = P0.tile([128, D], F32, tag="bvbc2")
                    r2 = P0.tile([1, D], F32, tag="bvr2")
                    nc.sync.dma_start(out=r2, in_=bv2)
                    nc.gpsimd.partition_broadcast(bvbc2, r2)

                    def kproj(wkey, rhs, bkey, ccin):
                        wt = wload(W[wkey], P_w0)
                        for do in range(NCH):
                            p = ps.tile([128, 512], F32, tag="u")
                            for ki in range(NCH):
                                mm(p, wt[ki][:, do * 128:(do + 1) * 128],
                                   rhs[ki], start=(ki == 0), stop=(ki == NCH - 1))
                            o = P0s.tile([128, 512], BF16, tag="kvo")
                            nc.scalar.activation(out=o, in_=p, func=AF.Identity,
                                                 bias=b_sb[bkey][:, do:do + 1], scale=1.0)
                            dst = ccin[do * 128 * 512:(do + 1) * 128 * 512]
                            nc.sync.dma_start(out=dst.rearrange("(p s) -> p s", s=512), in_=o)

                    def vproj(wkey, lhs, bvbc, ccin):
                        wt = wload(W[wkey], P_w0)
                        for sc in range(4):
                            p = ps.tile([128, D], F32, tag="u")
                            for ki in range(NCH):
                                for half in range(2):
                                    mm(p[:, half * 512:(half + 1) * 512],
                                       lhs[ki][:, sc * 128:(sc + 1) * 128],
                                       wt[ki][:, half * 512:(half + 1) * 512],
                                       start=(ki == 0), stop=(ki == NCH - 1))
                            o = P0s.tile([128, H, VW], BF16, tag="kvo2")
                            nc.vector.tensor_add(
                                o[:, :, 0:DK],
                                p.rearrange("p (h v) -> p h v", v=DK),
                                bvbc.rearrange("p (h v) -> p h v", v=DK))
                            nc.vector.tensor_copy(o[:, :, DK:VW], onesbc)
                            dst = ccin[KSEG + sc * 128 * H * VW:
                                       KSEG + (sc + 1) * 128 * H * VW]
                            nc.sync.dma_start(
                                out=dst.rearrange("(p h v) -> p h v", h=H, v=VW), in_=o)

                    kproj("k1", xk_t, "k1", CCIN1)
                    vproj("v1", xk_t, bvbc1, CCIN1)
                    nc.gpsimd.collective_compute(
                        "AllGather", mybir.AluOpType.bypass,
                        ins=[CCIN1], outs=[CCOUT1],
                        replica_groups=[[0, 1, 2, 3], [4, 5, 6, 7]],
                    )
                    kproj("k2", enc_t, "k2", CCIN2)
                    vproj("v2", enc_t, bvbc2, CCIN2)
                    nc.gpsimd.collective_compute(
                        "AllGather", mybir.AluOpType.bypass,
                        ins=[CCIN2], outs=[CCOUT2],
                        replica_groups=[[0, 1, 2, 3], [4, 5, 6, 7]],
                    )

                    # Q projection (overlaps the AllGathers)
                    for ki in range(NCH):
                        t = P_xq.tile([128, 512], F32R, tag=f"xq{ki}")
                        nc.sync.dma_start(out=t, in_=xqT[ki * 128:(ki + 1) * 128, :])
                        xq_t.append(t)
                    qT_t = [P_xq.tile([128, 512], BF16, tag=f"q{i}", name=f"qT{i}") for i in range(NCH)]
                    proj_feature(wload(W["q1"], P_w0), xq_t, b_sb["q1"], qT_t)

                # ---- shared attention ----
                def attention(qtiles, ccout, causal_, use_srcb, out_pairs, Pstr):
                    # V resident: one [128, H, VW] tile per k-block, shared by all pairs
                    vres = []
                    for kblk in range(NBLK):
                        r, lb = kblk // 4, kblk % 4
                        vt = Pstr.tile([128, H, VW], BF16, tag=f"vres{kblk}", bufs=1,
                                       name=f"vres{kblk}")
                        vsrc = ccout[r * SEG + KSEG + lb * 128 * H * VW:
                                     r * SEG + KSEG + (lb + 1) * 128 * H * VW]
                        nc.sync.dma_start(
                            out=vt, in_=vsrc.rearrange("(p h v) -> p h v", h=H, v=VW))
                        vres.append(vt)
                    for hp in range(NPAIR):
                        kt = Pstr.tile([128, 4, 512], BF16, tag="kt")
                        for r in range(4):
                            src = ccout[r * SEG + hp * 128 * 512:
                                        r * SEG + (hp + 1) * 128 * 512]
                            nc.sync.dma_start(out=kt[:, r, :],
                                               in_=src.rearrange("(p s) -> p s", s=512))
                        a0 = ps_att.tile([65, 512], F32, tag="a")
                        a1 = ps_att.tile([65, 512], F32, tag="a")
                        for kblk in range(NBLK):
                            sfx = _sfx(kblk, causal_)
                            if sfx >= 512:
                                continue
                            r, lb = kblk // 4, kblk % 4
                            vf = vres[kblk]

                            sc_ps = ps.tile([128, 2, 512], F32, tag="u")
                            for h in range(2):
                                bp = h * DK
                                mm(sc_ps[:, h, sfx:512],
                                   kt[bp:bp + DK, r, lb * 128:lb * 128 + 128],
                                   qtiles[hp][bp:bp + DK, sfx:512],
                                   start=True, stop=True, tile_position=(bp, 0))
                            es = Pstr.tile([128, 2, 512], BF16, tag="es")
                            if use_srcb:
                                nc.scalar.activation(out=es[:, :, sfx:512],
                                                     in_=sc_ps[:, :, sfx:512],
                                                     func=AF.Exp, scale=0.125,
                                                     bias=srcb_sb[:, kblk:kblk + 1])
                            else:
                                nc.scalar.activation(out=es[:, :, sfx:512],
                                                     in_=sc_ps[:, :, sfx:512],
                                                     func=AF.Exp, scale=0.125)
                            if causal_:
                                nc.vector.tensor_mul(
                                    es[:, :, sfx:sfx + 128],
                                    es[:, :, sfx:sfx + 128],
                                    msk_sb[:, kblk:kblk + 1, :].to_broadcast((128, 2, 128)))
                            first, last = (kblk == 0), (kblk == NBLK - 1)
                            mm(a0[:, sfx:512], vf[:, 2 * hp, :], es[:, 0, sfx:512],
                               start=first, stop=last, skip_group_check=True)
                            mm(a1[:, sfx:512], vf[:, 2 * hp + 1, :], es[:, 1, sfx:512],
                               start=first, stop=last, skip_group_check=True)
                        pair = out_pairs[hp]
                        for h, a in ((0, a0), (1, a1)):
                            srow = Pstr.tile([1, 512], F32, tag="srow")
                            nc.vector.tensor_copy(srow, a[64:65, :])
                            rec = Pstr.tile([1, 512], F32, tag="rec")
                            nc.vector.reciprocal_approx_fast(out=rec, in_=srow)
                            bc = Pstr.tile([128, 512], F32, tag="bc")
                            nc.gpsimd.partition_broadcast(bc[0:DK, :], rec)
                            nc.vector.tensor_mul(pair[h * DK:(h + 1) * DK, :],
                                                 a[0:DK, :], bc[0:DK, :])

                def ln(z_tiles, gkey, bkey, out_tiles, Pstr):
                    st0 = ps.tile([1, 512], F32, tag="u")
                    st1 = ps.tile([1, 512], F32, tag="u")
                    for k in range(NCH):
                        mm(st0, ones_t, z_tiles[k],
                           start=(k == 0), stop=(k == NCH - 1), skip_group_check=True)
                    zsq = []
                    for k in range(NCH):
                        t = Pstr.tile([128, 512], F32R, tag="zsq")
                        nc.vector.tensor_mul(t, z_tiles[k], z_tiles[k])
                        zsq.append(t)
                    for k in range(NCH):
                        mm(st1, ones_t, zsq[k],
                           start=(k == 0), stop=(k == NCH - 1), skip_group_check=True)
                    mean = Pstr.tile([1, 512], F32, tag="lnrow")
                    nc.vector.tensor_scalar_mul(mean, st0, 1.0 / D)
                    var = Pstr.tile([1, 512], F32, tag="lnrow")
                    nc.vector.tensor_scalar_mul(var, st1, 1.0 / D)
                    msq = Pstr.tile([1, 512], F32, tag="lnrow")
                    nc.vector.tensor_mul(msq, mean, mean)
                    nc.vector.tensor_sub(var, var, msq)
                    sd = Pstr.tile([1, 512], F32, tag="lnrow")
                    nc.scalar.activation(out=sd, in_=var, func=AF.Sqrt,
                                         bias=eps_t[0:1, :], scale=1.0)
                    rstd = Pstr.tile([1, 512], F32, tag="lnrow")
                    nc.vector.reciprocal_approx_fast(out=rstd, in_=sd)
                    nb = Pstr.tile([1, 512], F32, tag="lnrow")
                    nc.vector.tensor_mul(nb, mean, rstd)
                    nc.vector.tensor_scalar_mul(nb, nb, -1.0)
                    abc = Pstr.tile([128, 512], F32, tag="bc")
                    nc.gpsimd.partition_broadcast(abc, rstd)
                    bbc = Pstr.tile([128, 512], F32, tag="bc")
                    nc.gpsimd.partition_broadcast(bbc, nb)
                    for k in range(NCH):
                        t = Pstr.tile([128, 512], F32, tag="lnt")
                        nc.vector.tensor_mul(t, z_tiles[k], abc)
                        if affine:
                            t2 = Pstr.tile([128, 512], F32, tag="lnt")
                            nc.vector.tensor_add(t2, t, bbc)
                            nc.vector.tensor_scalar(out=out_tiles[k], in0=t2,
                                                    scalar1=gb_sb[gkey][:, k:k + 1],
                                                    scalar2=gb_sb[bkey][:, k:k + 1],
                                                    op0=mybir.AluOpType.mult,
                                                    op1=mybir.AluOpType.add)
                        else:
                            nc.vector.tensor_add(out_tiles[k], t, bbc)

                # deferred const loads (keep startup DMA lean)
                srcb_sb = P_const.tile([128, NBLK], F32, tag="srcb")
                nc.sync.dma_start(out=srcb_sb, in_=srcb)
                msk_sb = None
                if causal:
                    msk_sb = P_const.tile([128, NBLK, 128], BF16, tag="msk")
                    nc.sync.dma_start(out=msk_sb, in_=msk.rearrange("k p q -> p k q"))

                # =========== phase 1: self-attention + O1 + LN1 ===========
                attn_pairs = [P_y.tile([128, 512], BF16, tag=f"at{i}", name=f"atp{i}") for i in range(NPAIR)]
                y1_t = [P_y.tile([128, 512], F32R, tag=f"y{i}", name=f"y1t{i}") for i in range(NCH)]
                with tc.tile_pool(name="s1", bufs=3) as P_s1, \
                     tc.tile_pool(name="w1pool", bufs=9) as P_w1:
                    attention(qT_t, CCOUT1, causal, False, attn_pairs, P_s1)
                    wt = wload(W["o1"], P_w1)
                    for do in range(NCH):
                        p = ps.tile([128, 512], F32, tag="u")
                        for ki in range(NCH):
                            mm(p, wt[ki][:, do * 128:(do + 1) * 128],
                               attn_pairs[ki], start=(ki == 0), stop=(ki == NCH - 1))
                        o = P_s1.tile([128, 512], F32, tag="o1")
                        nc.scalar.activation(out=o, in_=p, func=AF.Identity,
                                             bias=b_sb["o1"][:, do:do + 1], scale=1.0)
                        # z1 in-place into xq tile (residual)
                        nc.vector.tensor_add(xq_t[do], o, xq_t[do])
                    ln(xq_t, "g1", "be1", y1_t, P_s1)

            # =========== phase 2: Q2 + cross-attention + O2 + LN2 ===========
            y2_t = [P_y.tile([128, 512], F32R, tag=f"y2{i}", name=f"y2t{i}") for i in range(NCH)]
            with tc.tile_pool(name="s2", bufs=3) as P_s2, \
                 tc.tile_pool(name="w2pool", bufs=9) as P_w2, \
                 tc.tile_pool(name="q2pool", bufs=1) as P_q2:
                q2_t = [P_q2.tile([128, 512], BF16, tag=f"qq{i}", name=f"q2t{i}") for i in range(NCH)]
                y1b = []
                for i in range(NCH):
                    t = P_q2.tile([128, 512], BF16, tag=f"y1b{i}", name=f"y1b{i}")
                    nc.vector.tensor_copy(t, y1_t[i])
                    y1b.append(t)
                proj_feature(wload(W["q2"], P_w2), y1b, b_sb["q2"], q2_t)
                attention(q2_t, CCOUT2, False, True, attn_pairs, P_s2)
                wt = wload(W["o2"], P_w2)
                for do in range(NCH):
                    p = ps.tile([128, 512], F32, tag="u")
                    for ki in range(NCH):
                        mm(p, wt[ki][:, do * 128:(do + 1) * 128],
                           attn_pairs[ki], start=(ki == 0), stop=(ki == NCH - 1))
                    o = P_s2.tile([128, 512], F32, tag="o2")
                    nc.scalar.activation(out=o, in_=p, func=AF.Identity,
                                         bias=b_sb["o2"][:, do:do + 1], scale=1.0)
                    nc.vector.tensor_add(y1_t[do], o, y1_t[do])  # z2 in-place
                ln(y1_t, "g2", "be2", y2_t, P_s2)

            # =========== phase 3: FFN + LN3 + output ===========
            with tc.tile_pool(name="s3", bufs=3) as P_s3, \
                 tc.tile_pool(name="hpool", bufs=2) as P_h, \
                 tc.tile_pool(name="wfpool", bufs=1) as P_wf, \
                 tc.tile_pool(name="holdpool", bufs=1) as P_hold:
                facc = [P_hold.tile([128, 512], F32, tag=f"fa{i}", name=f"facc{i}") for i in range(NCH)]
                y2b = []
                for i in range(NCH):
                    t = P_hold.tile([128, 512], BF16, tag=f"y2b{i}", name=f"y2b{i}")
                    nc.vector.tensor_copy(t, y2_t[i])
                    y2b.append(t)
                for g in range(8):  # groups of 4 df-chunks
                    w1g = []
                    for ki in range(NCH):
                        w = P_wf.tile([128, 512], BF16, tag="w1", name=f"w1g{ki}", bufs=12)
                        nc.gpsimd.dma_start(
                            out=w, in_=Wf1[ki * 128:(ki + 1) * 128, g * 512:(g + 1) * 512])
                        w1g.append(w)
                    hg = []
                    for j in range(4):
                        dfc = g * 4 + j
                        p = ps.tile([128, 512], F32, tag="u")
                        for ki in range(NCH):
                            mm(p, w1g[ki][:, j * 128:(j + 1) * 128], y2b[ki],
                               start=(ki == 0), stop=(ki == NCH - 1))
                        h = P_h.tile([128, 512], BF16, tag=f"h{j}")
                        nc.scalar.activation(out=h, in_=p, func=AF.Relu,
                                             bias=b_sb["f1"][:, dfc:dfc + 1], scale=1.0)
                        hg.append(h)
                    w2g = []
                    for j in range(4):
                        dfc = g * 4 + j
                        w = P_wf.tile([128, D], BF16, tag="w2", name=f"w2g{j}", bufs=6)
                        nc.gpsimd.dma_start(
                            out=w, in_=Wf2[dfc * 128:(dfc + 1) * 128, :])
                        w2g.append(w)
                    for do in range(NCH):
                        p2 = ps.tile([128, 512], F32, tag="u")
                        for j in range(4):
                            mm(p2, w2g[j][:, do * 128:(do + 1) * 128], hg[j],
                               start=(j == 0), stop=(j == 3))
                        if g == 0:
                            f = facc[do]
                            nc.vector.tensor_scalar_add(f, p2, b_sb["f2"][:, do:do + 1])
                        else:
                            nc.vector.tensor_add(facc[do], facc[do], p2)
                y3_t = [P_hold.tile([128, 512], F32, tag=f"y3{i}", name=f"y3t{i}") for i in range(NCH)]
                for do in range(NCH):
                    nc.vector.tensor_add(y2_t[do], facc[do], y2_t[do])  # z3 in-place
                ln(y2_t, "g3", "be3", y3_t, P_s3)
                for k in range(NCH):
                    nc.sync.dma_start(out=OUT[k * 128:(k + 1) * 128, :], in_=y3_t[k])

    nc.compile()
    return nc


def _get_nc(causal, affine):
    key = (causal, affine)
    if key not in _CACHE:
        _CACHE[key] = _build(causal, affine)
    return _CACHE[key]


def kernel(**inputs):
    inp = {k: np.asarray(v) for k, v in inputs.items()}
    x, enc = inp['x'].astype(np.float32), inp['enc_out'].astype(np.float32)
    tgt = np.asarray(inp['tgt_mask'])[0, 0]
    src = np.asarray(inp['src_mask'])
    causal = bool((tgt == np.tril(np.ones((S, S), tgt.dtype))).all())
    if not causal and not bool((tgt != 0).all()):
        raise NotImplementedError("tgt_mask must be causal-tril or all-ones")
    affine = not (all((inp[f'g{i}'] == 1).all() for i in (1, 2, 3))
                  and all((inp[f'be{i}'] == 0).all() for i in (1, 2, 3)))

    import ml_dtypes
    BF = ml_dtypes.bfloat16
    WT = {'q1': np.ascontiguousarray(inp['Wq1'].T)}
    for k in ['k1', 'v1', 'o1', 'q2', 'k2', 'v2', 'o2']:
        WT[k] = np.ascontiguousarray(inp['W' + k].T.astype(BF))
    WT['f1'] = np.ascontiguousarray(inp['Wf1'].T.astype(BF))
    WT['f2'] = np.ascontiguousarray(inp['Wf2'].T.astype(BF))
    bch = {k: np.ascontiguousarray(inp['b' + k].reshape(-1, 128).T)
           for k in ['q1', 'k1', 'o1', 'q2', 'k2', 'o2', 'f1', 'f2']}

    nc = _get_nc(causal, affine)

    in_maps = []
    for c in range(8):
        b, q = c // 4, c % 4
        qblocks = [q + 4 * j for j in range(4)]
        qrows = np.concatenate([np.arange(g * 128, g * 128 + 128) for g in qblocks])
        m = {
            'xqT': np.ascontiguousarray(x[b, qrows].T),
            'xkT': np.ascontiguousarray(x[b, q * 512:(q + 1) * 512].T.astype(BF)),
            'encT': np.ascontiguousarray(enc[b, q * 512:(q + 1) * 512].T.astype(BF)),
            'bv1': np.ascontiguousarray(inp['bv1'][None, :]),
            'bv2': np.ascontiguousarray(inp['bv2'][None, :]),
            'ones_in': np.ones((128, H, 1), ml_dtypes.bfloat16),
            'ones_f': np.ones((128, 1), np.float32),
            'srcb': np.ascontiguousarray(
                np.where(src[b, 0, 0] == 0, np.float32(-1e9), np.float32(0.0))
                .astype(np.float32).reshape(NBLK, 128).T),
        }
        for k in ['q1', 'k1', 'v1', 'o1', 'q2', 'k2', 'v2', 'o2']:
            m['W' + k] = WT[k]
        m['Wf1T'] = WT['f1']
        m['Wf2T'] = WT['f2']
        for k in ['q1', 'k1', 'o1', 'q2', 'k2', 'o2', 'f2', 'f1']:
            m['b' + k] = bch[k]
        if causal:
            ms = np.empty((NBLK, 128, 128), np.float32)
            for kblk in range(NBLK):
                gq = qblocks[kblk // 4]
                ms[kblk] = tgt[gq * 128:(gq + 1) * 128,
                               kblk * 128:(kblk + 1) * 128].T.astype(np.float32)
            m['mself'] = np.ascontiguousarray(ms.astype(ml_dtypes.bfloat16))
        if affine:
            for k in ['g1', 'be1', 'g2', 'be2', 'g3', 'be3']:
                m[k] = np.ascontiguousarray(inp[k].reshape(NCH, 128).T)
        in_maps.append(m)

    trace = bool(int(os.environ.get("KERNEL_TRACE", "0")))
    res = bass_utils.run_bass_kernel_spmd(
        nc, in_maps, core_ids=list(range(8)), trace=trace,
        tmpdir=(tempfile.mkdtemp(prefix="declayer_") if trace else None))
    kernel._last_results = res

    out = np.zeros((B, S, D), np.float32)
    for c in range(8):
        b, q = c // 4, c % 4
        qblocks = [q + 4 * j for j in range(4)]
        qrows = np.concatenate([np.arange(g * 128, g * 128 + 128) for g in qblocks])
        out[b, qrows] = res.results[c]['OUT'].T
    return out


# revision 34
# speedup vs baseline: 1.9708x; 1.0555x over previous
"""Trainium2 Bass kernel for nn_DecoderLayer (self-attn + cross-attn + FFN, 3 LNs).

Sharding: 8 cores = 2 batches x 4 query-shards. Core c handles batch c//4 and
query blocks {q, q+4, q+8, q+12} (q = c%4, blocks of 128 rows) — stride-4 for
causal load balance with a padded-uniform suffix structure so all cores run the
same SPMD program. K/V projections are computed on contiguous 512-row shards
and exchanged with a single AllGather (self KV + cross KV together).

Layouts: activations feature-major (x.T: [d, seq] with d on partitions);
V position-major ([seq, dv]) so attn@V needs no transposes; scores computed
transposed ([kpos, q]) with softmax sums taken via an appended ones-column in
the V matmul. All matmuls run in float32r (full PE rate, ~1.5e-4 rel err).
"""
import os
import sys
import tempfile

import numpy as np

sys.path.insert(0, '/opt/trn_rl_repo')

import concourse.mybir as mybir  # noqa: E402
import concourse.tile as tile  # noqa: E402
from concourse import bacc, bass_utils  # noqa: E402

B, S, T, D, H, DK, DF = 2, 2048, 2048, 1024, 16, 64, 4096
EPS = 1e-5
NBLK = S // 128          # 16 k-blocks
NCH = D // 128           # 8 feature chunks
NPAIR = H // 2           # 8 head pairs
VW = DK + 1              # V row width per head (ones column baked in)
HH = H // 2              # heads per AG half
KSEGH = 512 * 512        # K half: 4 do-chunks x [128, 512]
VSEGH = 512 * HH * VW    # V half: [512 s, 8 heads, 65]
SEGH = KSEGH + VSEGH     # per-rank elements of one half-AllGather

F32 = mybir.dt.float32
F32R = mybir.dt.float32r
BF16 = mybir.dt.bfloat16
AF = mybir.ActivationFunctionType

_CACHE = {}


def _R(ap):
    return ap.bitcast(F32R) if ap.dtype == F32 else ap


def _sfx(kblk, causal):
    return 128 * (kblk // 4) if causal else 0


def _build(causal, affine):
    nc = bacc.Bacc("TRN2", target_bir_lowering=False, debug=False, num_devices=8)

    def mm(out, lhsT, rhs, **kw):
        nc.tensor.matmul(out, _R(lhsT), _R(rhs), **kw)

    def din(name, shape, dtype=F32):
        return nc.dram_tensor(name, shape, dtype, kind="ExternalInput").ap()

    xqT = din("xqT", [D, 512], F32R)
    xkT = din("xkT", [D, 512], BF16)
    encT = din("encT", [D, 512], BF16)
    W = {k: din("W" + k, [D, D], BF16)
         for k in ["q1", "k1", "v1", "o1", "q2", "k2", "v2", "o2"]}
    Wf1 = din("Wf1T", [D, DF], BF16)
    Wf2 = din("Wf2T", [DF, D], BF16)
    bias_in = {k: din("b" + k, [128, NCH]) for k in ["q1", "k1", "o1", "q2", "k2", "o2", "f2"]}
    bias_in["f1"] = din("bf1", [128, DF // 128])
    bv1 = din("bv1", [1, D])
    bv2 = din("bv2", [1, D])
    srcb = din("srcb", [128, NBLK])
    ones_in = din("ones_in", [128, H, 1], BF16)
    ones_f = din("ones_f", [128, 1], F32R)
    if causal:
        msk = din("mself", [NBLK, 128, 128], BF16)
    gb = {}
    if affine:
        for k in ["g1", "be1", "g2", "be2", "g3", "be3"]:
            gb[k] = din(k, [128, NCH])
    OUT = nc.dram_tensor("OUT", [D, 512], F32, kind="ExternalOutput").ap()

    CCIN = {}
    CCOUT = {}
    for nm in ["sa", "sb", "ca", "cb"]:
        CCIN[nm] = nc.dram_tensor("ccin_" + nm, [SEGH], BF16).ap()
        CCOUT[nm] = nc.dram_tensor("ccout_" + nm, [4 * SEGH], BF16).ap()

    with tile.TileContext(nc) as tc:
        with tc.tile_pool(name="const", bufs=1) as P_const, \
             tc.tile_pool(name="ps", bufs=3, space="PSUM") as ps, \
             tc.tile_pool(name="psatt", bufs=2, space="PSUM") as ps_att, \
             tc.tile_pool(name="ypool", bufs=1) as P_y:

            # ---- constants ----
            ones_t = P_const.tile([128, 1], F32R, tag="ones")
            nc.sync.dma_start(out=ones_t, in_=ones_f)
            eps_t = P_const.tile([128, 1], F32, tag="eps")
            nc.vector.memset(eps_t, EPS)
            b_sb = {}
            for k, ap_ in bias_in.items():
                t = P_const.tile(list(ap_.shape), F32, tag="b" + k)
                nc.sync.dma_start(out=t, in_=ap_)
                b_sb[k] = t
            gb_sb = {}
            if affine:
                for k in gb:
                    t = P_const.tile([128, NCH], F32, tag=k)
                    nc.sync.dma_start(out=t, in_=gb[k])
                    gb_sb[k] = t

            def wload(Wap, pool):
                tiles = []
                for ki in range(NCH):
                    t = pool.tile([128, Wap.shape[1]], Wap.dtype, tag="w", name=f"wt{ki}")
                    nc.gpsimd.dma_start(out=t, in_=Wap[ki * 128:(ki + 1) * 128, :])
                    tiles.append(t)
                return tiles

            def proj_feature(wt, rhs_tiles, bias_t, out_tiles, act=AF.Identity):
                for do in range(NCH):
                    p = ps.tile([128, 512], F32, tag="u")
                    for ki in range(NCH):
                        mm(p, wt[ki][:, do * 128:(do + 1) * 128],
                           rhs_tiles[ki], start=(ki == 0), stop=(ki == NCH - 1))
                    nc.scalar.activation(out=out_tiles[do], in_=p, func=act,
                                         bias=bias_t[:, do:do + 1], scale=1.0)

            # =========== phase 0: KV projections + AllGather + Q ===========
            with tc.tile_pool(name="xqpool", bufs=1) as P_xq:
                xq_t = []
                qT_t = []
                with tc.tile_pool(name="p0", bufs=1) as P0, \
                     tc.tile_pool(name="p0w", bufs=18) as P_w0, \
                     tc.tile_pool(name="p0s", bufs=3) as P0s:
                    onesbc = P0.tile([128, H, 1], BF16, tag="onesbc")
                    nc.sync.dma_start(out=onesbc, in_=ones_in)
                    xk_t, enc_t = [], []
                    for ki in range(NCH):
                        t = P0.tile([128, 512], BF16, tag=f"xk{ki}")
                        nc.sync.dma_start(out=t, in_=xkT[ki * 128:(ki + 1) * 128, :])
                        xk_t.append(t)
                    for ki in range(NCH):
                        t = P0.tile([128, 512], BF16, tag=f"en{ki}")
                        nc.sync.dma_start(out=t, in_=encT[ki * 128:(ki + 1) * 128, :])
                        enc_t.append(t)
                    bvbc1 = P0.tile([128, D], F32, tag="bvbc1")
                    r1 = P0.tile([1, D], F32, tag="bvr1")
                    nc.sync.dma_start(out=r1, in_=bv1)
                    nc.gpsimd.partition_broadcast(bvbc1, r1)
                    bvbc2 = P0.tile([128, D], F32, tag="bvbc2")
                    r2 = P0.tile([1, D], F32, tag="bvr2")
                    nc.sync.dma_start(out=r2, in_=bv2)
                    nc.gpsimd.partition_broadcast(bvbc2, r2)

                    def kproj_half(wt, rhs, bkey, ccin, half):
                        for j in range(4):
                            do = half * 4 + j
                            p = ps.tile([128, 512], F32, tag="u")
                            for ki in range(NCH):
                                mm(p, wt[ki][:, do * 128:(do + 1) * 128],
                                   rhs[ki], start=(ki == 0), stop=(ki == NCH - 1))
                            o = P0s.tile([128, 512], BF16, tag="kvo")
                            nc.scalar.activation(out=o, in_=p, func=AF.Identity,
                                                 bias=b_sb[bkey][:, do:do + 1], scale=1.0)
                            dst = ccin[j * 128 * 512:(j + 1) * 128 * 512]
                            nc.sync.dma_start(out=dst.rearrange("(p s) -> p s", s=512), in_=o)

                    def vproj_half(wt, lhs, bvbc, ccin, half):
                        for sc in range(4):
                            p = ps.tile([128, 512], F32, tag="u")
                            for ki in range(NCH):
                                mm(p, lhs[ki][:, sc * 128:(sc + 1) * 128],
                                   wt[ki][:, half * 512:(half + 1) * 512],
                                   start=(ki == 0), stop=(ki == NCH - 1))
                            o = P0s.tile([128, HH, VW], BF16, tag="kvo2")
                            nc.vector.tensor_add(
                                o[:, :, 0:DK],
                                p.rearrange("p (h v) -> p h v", v=DK),
                                bvbc.rearrange("p (h v) -> p h v", v=DK)[:, half * HH:(half + 1) * HH, :])
                            nc.vector.tensor_copy(o[:, :, DK:VW], onesbc[:, 0:HH, :])
                            dst = ccin[KSEGH + sc * 128 * HH * VW:
                                       KSEGH + (sc + 1) * 128 * HH * VW]
                            nc.sync.dma_start(
                                out=dst.rearrange("(p h v) -> p h v", h=HH, v=VW), in_=o)

                    def fire_ag(nm):
                        nc.gpsimd.collective_compute(
                            "AllGather", mybir.AluOpType.bypass,
                            ins=[CCIN[nm]], outs=[CCOUT[nm]],
                            replica_groups=[[0, 1, 2, 3], [4, 5, 6, 7]],
                        )

                    wk1 = wload(W["k1"], P_w0)
                    wv1 = wload(W["v1"], P_w0)
                    kproj_half(wk1, xk_t, "k1", CCIN["sa"], 0)
                    vproj_half(wv1, xk_t, bvbc1, CCIN["sa"], 0)
                    fire_ag("sa")
                    kproj_half(wk1, xk_t, "k1", CCIN["sb"], 1)
                    vproj_half(wv1, xk_t, bvbc1, CCIN["sb"], 1)
                    fire_ag("sb")
                    wk2 = wload(W["k2"], P_w0)
                    wv2 = wload(W["v2"], P_w0)
                    kproj_half(wk2, enc_t, "k2", CCIN["ca"], 0)
                    vproj_half(wv2, enc_t, bvbc2, CCIN["ca"], 0)
                    fire_ag("ca")
                    kproj_half(wk2, enc_t, "k2", CCIN["cb"], 1)
                    vproj_half(wv2, enc_t, bvbc2, CCIN["cb"], 1)
                    fire_ag("cb")

                    # Q projection (overlaps the AllGathers)
                    xqb = []
                    for ki in range(NCH):
                        t = P_xq.tile([128, 512], F32R, tag=f"xq{ki}")
                        nc.sync.dma_start(out=t, in_=xqT[ki * 128:(ki + 1) * 128, :])
                        xq_t.append(t)
                        tb = P0.tile([128, 512], BF16, tag=f"xqb{ki}", name=f"xqb{ki}")
                        nc.vector.tensor_copy(tb, t)
                        xqb.append(tb)
                    qT_t = [P_xq.tile([128, 512], BF16, tag=f"q{i}", name=f"qT{i}") for i in range(NCH)]
                    proj_feature(wload(W["q1"], P_w0), xqb, b_sb["q1"], qT_t)

                # ---- shared attention ----
                def attention(qtiles, cc_a, cc_b, causal_, use_srcb, out_pairs, Pstr):
                    # V resident: per (k-block, half) [128, HH, VW], shared by all pairs
                    vres = [[None, None] for _ in range(NBLK)]
                    for half, cc in ((0, cc_a), (1, cc_b)):
                        for kblk in range(NBLK):
                            r, lb = kblk // 4, kblk % 4
                            vt = Pstr.tile([128, HH, VW], BF16, bufs=1,
                                           tag=f"vres{kblk}h{half}",
                                           name=f"vres{kblk}h{half}")
                            vsrc = cc[r * SEGH + KSEGH + lb * 128 * HH * VW:
                                      r * SEGH + KSEGH + (lb + 1) * 128 * HH * VW]
                            nc.sync.dma_start(
                                out=vt, in_=vsrc.rearrange("(p h v) -> p h v", h=HH, v=VW))
                            vres[kblk][half] = vt
                    for hp in range(NPAIR):
                        half, hl = hp // 4, hp % 4
                        cc = cc_a if half == 0 else cc_b
                        kt = Pstr.tile([128, 4, 512], BF16, tag="kt")
                        for r in range(4):
                            src = cc[r * SEGH + hl * 128 * 512:
                                     r * SEGH + (hl + 1) * 128 * 512]
                            nc.sync.dma_start(out=kt[:, r, :],
                                               in_=src.rearrange("(p s) -> p s", s=512))
                        a0 = ps_att.tile([65, 512], F32, tag="a")
                        a1 = ps_att.tile([65, 512], F32, tag="a")
                        for kblk in range(NBLK):
                            sfx = _sfx(kblk, causal_)
                            if sfx >= 512:
                                continue
                            vf = vres[kblk][half]

                            r, lb = kblk // 4, kblk % 4
                            sc_ps = ps.tile([128, 2, 512], F32, tag="u")
                            for h in range(2):
                                bp = h * DK
                                mm(sc_ps[:, h, sfx:512],
                                   kt[bp:bp + DK, r, lb * 128:lb * 128 + 128],
                                   qtiles[hp][bp:bp + DK, sfx:512],
                                   start=True, stop=True, tile_position=(bp, 0))
                            es = Pstr.tile([128, 2, 512], BF16, tag="es")
                            if use_srcb:
                                nc.scalar.activation(out=es[:, :, sfx:512],
                                                     in_=sc_ps[:, :, sfx:512],
                                                     func=AF.Exp, scale=0.125,
                                                     bias=srcb_sb[:, kblk:kblk + 1])
                            else:
                                nc.scalar.activation(out=es[:, :, sfx:512],
                                                     in_=sc_ps[:, :, sfx:512],
                                                     func=AF.Exp, scale=0.125)
                            if causal_:
                                nc.vector.tensor_mul(
                                    es[:, :, sfx:sfx + 128],
                                    es[:, :, sfx:sfx + 128],
                                    msk_sb[:, kblk:kblk + 1, :].to_broadcast((128, 2, 128)))
                            first, last = (kblk == 0), (kblk == NBLK - 1)
                            mm(a0[:, sfx:512], vf[:, 2 * hl, :], es[:, 0, sfx:512],
                               start=first, stop=last, skip_group_check=True)
                            mm(a1[:, sfx:512], vf[:, 2 * hl + 1, :], es[:, 1, sfx:512],
                               start=first, stop=last, skip_group_check=True)
                        pair = out_pairs[hp]
                        for h, a in ((0, a0), (1, a1)):
                            srow = Pstr.tile([1, 512], F32, tag="srow")
                            nc.vector.tensor_copy(srow, a[64:65, :])
                            rec = Pstr.tile([1, 512], F32, tag="rec")
                            nc.vector.reciprocal_approx_fast(out=rec, in_=srow)
                            bc = Pstr.tile([128, 512], F32, tag="bc")
                            nc.gpsimd.partition_broadcast(bc[0:DK, :], rec)
                            nc.vector.tensor_mul(pair[h * DK:(h + 1) * DK, :],
                                                 a[0:DK, :], bc[0:DK, :])

                def ln(z_tiles, gkey, bkey, out_tiles, Pstr):
                    st0 = ps.tile([1, 512], F32, tag="u")
                    st1 = ps.tile([1, 512], F32, tag="u")
                    for k in range(NCH):
                        mm(st0, ones_t, z_tiles[k],
                           start=(k == 0), stop=(k == NCH - 1), skip_group_check=True)
                    zsq = []
                    for k in range(NCH):
                        t = Pstr.tile([128, 512], F32R, tag="zsq")
                        nc.vector.tensor_mul(t, z_tiles[k], z_tiles[k])
                        zsq.append(t)
                    for k in range(NCH):
                        mm(st1, ones_t, zsq[k],
                           start=(k == 0), stop=(k == NCH - 1), skip_group_check=True)
                    mean = Pstr.tile([1, 512], F32, tag="lnrow")
                    nc.vector.tensor_scalar_mul(mean, st0, 1.0 / D)
                    var = Pstr.tile([1, 512], F32, tag="lnrow")
                    nc.vector.tensor_scalar_mul(var, st1, 1.0 / D)
                    msq = Pstr.tile([1, 512], F32, tag="lnrow")
                    nc.vector.tensor_mul(msq, mean, mean)
                    nc.vector.tensor_sub(var, var, msq)
                    sd = Pstr.tile([1, 512], F32, tag="lnrow")
                    nc.scalar.activation(out=sd, in_=var, func=AF.Sqrt,
                                         bias=eps_t[0:1, :], scale=1.0)
                    rstd = Pstr.tile([1, 512], F32, tag="lnrow")
                    nc.vector.reciprocal_approx_fast(out=rstd, in_=sd)
                    nb = Pstr.tile([1, 512], F32, tag="lnrow")
                    nc.vector.tensor_mul(nb, mean, rstd)
                    nc.vector.tensor_scalar_mul(nb, nb, -1.0)
                    abc = Pstr.tile([128, 512], F32, tag="bc")
                    nc.gpsimd.partition_broadcast(abc, rstd)
                    bbc = Pstr.tile([128, 512], F32, tag="bc")
                    nc.gpsimd.partition_broadcast(bbc, nb)
                    for k in range(NCH):
                        t = Pstr.tile([128, 512], F32, tag="lnt")
                        nc.vector.tensor_mul(t, z_tiles[k], abc)
                        if affine:
                            t2 = Pstr.tile([128, 512], F32, tag="lnt")
                            nc.vector.tensor_add(t2, t, bbc)
                            nc.vector.tensor_scalar(out=out_tiles[k], in0=t2,
                                                    scalar1=gb_sb[gkey][:, k:k + 1],
                                                    scalar2=gb_sb[bkey][:, k:k + 1],
                                                    op0=mybir.AluOpType.mult,
                                                    op1=mybir.AluOpType.add)
                        else:
                            nc.vector.tensor_add(out_tiles[k], t, bbc)

                # deferred const loads (keep startup DMA lean)
                srcb_sb = P_const.tile([128, NBLK], F32, tag="srcb")
                nc.sync.dma_start(out=srcb_sb, in_=srcb)
                msk_sb = None
                if causal:
                    msk_sb = P_const.tile([128, NBLK, 128], BF16, tag="msk")
                    nc.sync.dma_start(out=msk_sb, in_=msk.rearrange("k p q -> p k q"))

                # =========== phase 1: self-attention + O1 + LN1 ===========
                attn_pairs = [P_y.tile([128, 512], BF16, tag=f"at{i}", name=f"atp{i}") for i in range(NPAIR)]
                y1_t = [P_y.tile([128, 512], F32R, tag=f"y{i}", name=f"y1t{i}") for i in range(NCH)]
                with tc.tile_pool(name="s1", bufs=3) as P_s1, \
                     tc.tile_pool(name="w1pool", bufs=9) as P_w1:
                    attention(qT_t, CCOUT["sa"], CCOUT["sb"], causal, False, attn_pairs, P_s1)
                    wt = wload(W["o1"], P_w1)
                    for do in range(NCH):
                        p = ps.tile([128, 512], F32, tag="u")
                        for ki in range(NCH):
                            mm(p, wt[ki][:, do * 128:(do + 1) * 128],
                               attn_pairs[ki], start=(ki == 0), stop=(ki == NCH - 1))
                        o = P_s1.tile([128, 512], F32, tag="o1")
                        nc.scalar.activation(out=o, in_=p, func=AF.Identity,
                                             bias=b_sb["o1"][:, do:do + 1], scale=1.0)
                        # z1 in-place into xq tile (residual)
                        nc.vector.tensor_add(xq_t[do], o, xq_t[do])
                    ln(xq_t, "g1", "be1", y1_t, P_s1)

            # =========== phase 2: Q2 + cross-attention + O2 + LN2 ===========
            y2_t = [P_y.tile([128, 512], F32R, tag=f"y2{i}", name=f"y2t{i}") for i in range(NCH)]
            with tc.tile_pool(name="s2", bufs=3) as P_s2, \
                 tc.tile_pool(name="w2pool", bufs=9) as P_w2, \
                 tc.tile_pool(name="q2pool", bufs=1) as P_q2:
                q2_t = [P_q2.tile([128, 512], BF16, tag=f"qq{i}", name=f"q2t{i}") for i in range(NCH)]
                y1b = []
                for i in range(NCH):
                    t = P_q2.tile([128, 512], BF16, tag=f"y1b{i}", name=f"y1b{i}")
                    nc.vector.tensor_copy(t, y1_t[i])
                    y1b.append(t)
                proj_feature(wload(W["q2"], P_w2), y1b, b_sb["q2"], q2_t)
                attention(q2_t, CCOUT["ca"], CCOUT["cb"], False, True, attn_pairs, P_s2)
                wt = wload(W["o2"], P_w2)
                for do in range(NCH):
                    p = ps.tile([128, 512], F32, tag="u")
                    for ki in range(NCH):
                        mm(p, wt[ki][:, do * 128:(do + 1) * 128],
                           attn_pairs[ki], start=(ki == 0), stop=(ki == NCH - 1))
                    o = P_s2.tile([128, 512], F32, tag="o2")
                    nc.scalar.activation(out=o, in_=p, func=AF.Identity,
                                         bias=b_sb["o2"][:, do:do + 1], scale=1.0)
                    nc.vector.tensor_add(y1_t[do], o, y1_t[do])  # z2 in-place
                ln(y1_t, "g2", "be2", y2_t, P_s2)

            # =========== phase 3: FFN + LN3 + output ===========
            with tc.tile_pool(name="s3", bufs=3) as P_s3, \
                 tc.tile_pool(name="hpool", bufs=2) as P_h, \
                 tc.tile_pool(name="wfpool", bufs=1) as P_wf, \
                 tc.tile_pool(name="holdpool", bufs=1) as P_hold:
                facc = [P_hold.tile([128, 512], F32, tag=f"fa{i}", name=f"facc{i}") for i in range(NCH)]
                y2b = []
                for i in range(NCH):
                    t = P_hold.tile([128, 512], BF16, tag=f"y2b{i}", name=f"y2b{i}")
                    nc.vector.tensor_copy(t, y2_t[i])
                    y2b.append(t)
                for g in range(8):  # groups of 4 df-chunks
                    w1g = []
                    for ki in range(NCH):
                        w = P_wf.tile([128, 512], BF16, tag="w1", name=f"w1g{ki}", bufs=12)
                        nc.gpsimd.dma_start(
                            out=w, in_=Wf1[ki * 128:(ki + 1) * 128, g * 512:(g + 1) * 512])
                        w1g.append(w)
                    hg = []
                    for j in range(4):
                        dfc = g * 4 + j
                        p = ps.tile([128, 512], F32, tag="u")
                        for ki in range(NCH):
                            mm(p, w1g[ki][:, j * 128:(j + 1) * 128], y2b[ki],
                               start=(ki == 0), stop=(ki == NCH - 1))
                        h = P_h.tile([128, 512], BF16, tag=f"h{j}")
                        nc.scalar.activation(out=h, in_=p, func=AF.Relu,
                                             bias=b_sb["f1"][:, dfc:dfc + 1], scale=1.0)
                        hg.append(h)
                    w2g = []
                    for j in range(4):
                        dfc = g * 4 + j
                        w = P_wf.tile([128, D], BF16, tag="w2", name=f"w2g{j}", bufs=6)
                        nc.gpsimd.dma_start(
                            out=w, in_=Wf2[dfc * 128:(dfc + 1) * 128, :])
                        w2g.append(w)
                    for do in range(NCH):
                        p2 = ps.tile([128, 512], F32, tag="u")
                        for j in range(4):
                            mm(p2, w2g[j][:, do * 128:(do + 1) * 128], hg[j],
                               start=(j == 0), stop=(j == 3))
                        if g == 0:
                            f = facc[do]
                            nc.vector.tensor_scalar_add(f, p2, b_sb["f2"][:, do:do + 1])
                        else:
                            nc.vector.tensor_add(facc[do], facc[do], p2)
                y3_t = [P_hold.tile([128, 512], F32, tag=f"y3{i}", name=f"y3t{i}") for i in range(NCH)]
                for do in range(NCH):
                    nc.vector.tensor_add(y2_t[do], facc[do], y2_t[do])  # z3 in-place
                ln(y2_t, "g3", "be3", y3_t, P_s3)
                for k in range(NCH):
                    nc.sync.dma_start(out=OUT[k * 128:(k + 1) * 128, :], in_=y3_t[k])

    nc.compile()
    return nc


def _get_nc(causal, affine):
    key = (causal, affine)
    if key not in _CACHE:
        _CACHE[key] = _build(causal, affine)
    return _CACHE[key]


def kernel(**inputs):
    inp = {k: np.asarray(v) for k, v in inputs.items()}
    x, enc = inp['x'].astype(np.float32), inp['enc_out'].astype(np.float32)
    tgt = np.asarray(inp['tgt_mask'])[0, 0]
    src = np.asarray(inp['src_mask'])
    causal = bool((tgt == np.tril(np.ones((S, S), tgt.dtype))).all())
    if not causal and not bool((tgt != 0).all()):
        raise NotImplementedError("tgt_mask must be causal-tril or all-ones")
    affine = not (all((inp[f'g{i}'] == 1).all() for i in (1, 2, 3))
                  and all((inp[f'be{i}'] == 0).all() for i in (1, 2, 3)))

    import ml_dtypes
    BF = ml_dtypes.bfloat16
    WT = {}
    for k in ['q1', 'k1', 'v1', 'o1', 'q2', 'k2', 'v2', 'o2']:
        WT[k] = np.ascontiguousarray(inp['W' + k].T.astype(BF))
    WT['f1'] = np.ascontiguousarray(inp['Wf1'].T.astype(BF))
    WT['f2'] = np.ascontiguousarray(inp['Wf2'].T.astype(BF))
    bch = {k: np.ascontiguousarray(inp['b' + k].reshape(-1, 128).T)
           for k in ['q1', 'k1', 'o1', 'q2', 'k2', 'o2', 'f1', 'f2']}

    nc = _get_nc(causal, affine)

    in_maps = []
    for c in range(8):
        b, q = c // 4, c % 4
        qblocks = [q + 4 * j for j in range(4)]
        qrows = np.concatenate([np.arange(g * 128, g * 128 + 128) for g in qblocks])
        m = {
            'xqT': np.ascontiguousarray(x[b, qrows].T),
            'xkT': np.ascontiguousarray(x[b, q * 512:(q + 1) * 512].T.astype(BF)),
            'encT': np.ascontiguousarray(enc[b, q * 512:(q + 1) * 512].T.astype(BF)),
            'bv1': np.ascontiguousarray(inp['bv1'][None, :]),
            'bv2': np.ascontiguousarray(inp['bv2'][None, :]),
            'ones_in': np.ones((128, H, 1), ml_dtypes.bfloat16),
            'ones_f': np.ones((128, 1), np.float32),
            'srcb': np.ascontiguousarray(
                np.where(src[b, 0, 0] == 0, np.float32(-1e9), np.float32(0.0))
                .astype(np.float32).reshape(NBLK, 128).T),
        }
        for k in ['q1', 'k1', 'v1', 'o1', 'q2', 'k2', 'v2', 'o2']:
            m['W' + k] = WT[k]
        m['Wf1T'] = WT['f1']
        m['Wf2T'] = WT['f2']
        for k in ['q1', 'k1', 'o1', 'q2', 'k2', 'o2', 'f2', 'f1']:
            m['b' + k] = bch[k]
        if causal:
            ms = np.empty((NBLK, 128, 128), np.float32)
            for kblk in range(NBLK):
                gq = qblocks[kblk // 4]
                ms[kblk] = tgt[gq * 128:(gq + 1) * 128,
                               kblk * 128:(kblk + 1) * 128].T.astype(np.float32)
            m['mself'] = np.ascontiguousarray(ms.astype(ml_dtypes.bfloat16))
        if affine:
            for k in ['g1', 'be1', 'g2', 'be2', 'g3', 'be3']:
                m[k] = np.ascontiguousarray(inp[k].reshape(NCH, 128).T)
        in_maps.append(m)

    trace = bool(int(os.environ.get("KERNEL_TRACE", "0")))
    res = bass_utils.run_bass_kernel_spmd(
        nc, in_maps, core_ids=list(range(8)), trace=trace,
        tmpdir=(tempfile.mkdtemp(prefix="declayer_") if trace else None))
    kernel._last_results = res

    out = np.zeros((B, S, D), np.float32)
    for c in range(8):
        b, q = c // 4, c % 4
        qblocks = [q + 4 * j for j in range(4)]
        qrows = np.concatenate([np.arange(g * 128, g * 128 + 128) for g in qblocks])
        out[b, qrows] = res.results[c]['OUT'].T
    return out
